# revision 1
# baseline (speedup 1.0000x reference)
"""GCN2 (nn_GCN2_42331197669873) Bass kernel for 8 TRN2 NeuronCores.

Strategy: graph/data parallel. Nodes sharded row-wise across 8 cores
(12500 each). Per layer:
  - AllGather the node features into a full replica in each core's HBM.
  - Sparse propagate: edges bucketed by destination core; within a core,
    edges are assigned to 14 fixed source-windows (7143 rows each) so the
    SWDGE dma_gather can address rows with int16 indices relative to a
    compile-time window base shared by all cores (SPMD).  Gathered rows are
    multiplied by edge weights (per-partition scalars on the ACT engine)
    and scatter-added into per-core HBM aggregation buffers with
    dma_scatter_add.  Duplicate dst indices inside one scatter call lose
    adds on HW (read-modify-writes pipeline), so tokens in each batch are
    grouped into occurrence-runs: run 1 (first occurrence of each dst) goes
    to buffer A, runs >=2 go to buffer B; calls targeting the same buffer
    serialize via Tile's WAW tracking, different buffers run in parallel.
  - Class-center branch: centers = sum_cores (D_shard^T @ (x + 0.1 x0))
    with D = onehot(label)/cnt precomputed on host, tiny [47,128]
    AllReduce, then a Gram-matrix formulation that never materializes the
    [47,47,128] pair tensor: rcm_i = rowsum(a)_i * c_i - (a @ c)_i with
    a = cm / pairdist.
  - Dense phase in feature-major layout (h on partitions) so conv / p@r
    matmuls stream node tiles against stationary weights.

kernel(**inputs) takes the FULL unsharded inputs and returns the FULL
[100000, 47] output; sharding + all preprocessing happens on host inside.
"""

import math
import numpy as np

from concourse import bass, bacc, tile, mybir, bass_utils
from concourse import library_config
from concourse.mybir import AxisListType
import concourse.tile_sem_assignment as _tsa
from concourse import bass_isa as _bisa

# Tile round-robins Pool-engine DMAs over all DMASW sem lanes ignoring
# queue_num; mixing SWDGE queues on one lane breaks its in-order-completion
# assumption (sim: "sem locked to SWDGE queue"). Segregate lanes by queue:
# queue 0 -> lanes [0,4), queue 1 -> lanes [4,8).
_orig_assign_tick = _tsa.TileClockTick._assign_tick

def _assign_tick_qsplit(self, inst):
    if (isinstance(inst, _tsa.DMAInst)
            and inst.engine == mybir.EngineType.Pool
            and not isinstance(inst, _bisa.UserSyncedRemoteDMADescs)
            and self.swdge_sem_count >= 2):
        qn = getattr(inst, "queue_num", 0) or 0
        half = self.swdge_sem_count // 2
        if not hasattr(self, "_qrr"):
            self._qrr = {}
        r = self._qrr.get(qn, 0)
        self._qrr[qn] = r + 1
        self.next_sw_dma_idx = (qn % 2) * half + r % half
    return _orig_assign_tick(self, inst)

_tsa.TileClockTick._assign_tick = _assign_tick_qsplit

F32 = mybir.dt.float32
BF16 = mybir.dt.bfloat16
I16 = mybir.dt.int16


class Cfg:
    def __init__(self, N=100000, E=800000, C=47, H=128, ncores=8, nbatch=14,
                 nb_tok=8192, L=4, alpha=0.1, theta=0.5, rsl=0.5):
        self.N, self.E, self.C, self.H = N, E, C, H
        self.ncores = ncores
        self.NS = N // ncores                 # nodes per core
        self.NT = (self.NS + 127) // 128      # node tiles per core
        self.nbatch = nbatch                  # src windows
        self.W = (N + nbatch - 1) // nbatch   # window width (int16-safe rel idx)
        assert self.W + 256 < 32768
        self.NB = nb_tok                      # tokens per batch (128-mult)
        self.L, self.alpha, self.theta, self.rsl = L, alpha, theta, rsl
        self.trash = self.NT * 128            # scatter trash row (pads)
        self.nagg = self.NT * 128 + 128       # agg rows incl trash row


DEF = Cfg()


# ----------------------------------------------------------------------
# host-side edge preprocessing
# ----------------------------------------------------------------------

def _prep_edges(cfg, edge_index, edge_weight):
    """Per core: batch/window/occurrence-run token layout.

    Returns gidx [nc, nbatch, 128, NB//16] i16, sidx (same), wgt
    [nc, nbatch, 128, NB//128] f32, runs: list of (c0, c1, buf) column
    ranges shared by all cores.
    """
    src = np.asarray(edge_index[0], np.int64)
    dst = np.asarray(edge_index[1], np.int64)
    w = np.asarray(edge_weight, np.float32)
    nc, NS, NB, nbatch, W = cfg.ncores, cfg.NS, cfg.NB, cfg.nbatch, cfg.W

    # token lists per (core, batch, occ-class)
    per_cb = [[None] * nbatch for _ in range(nc)]
    kmax_all = 1
    for c in range(nc):
        m = (dst >= c * NS) & (dst < (c + 1) * NS)
        s_c, d_c, w_c = src[m], dst[m] - c * NS, w[m]
        b_c = s_c // W
        for b in range(nbatch):
            mb = b_c == b
            s_b, d_b, w_b = s_c[mb], d_c[mb], w_c[mb]
            # occurrence index per dst within the batch
            order = np.argsort(d_b, kind="stable")
            s_b, d_b, w_b = s_b[order], d_b[order], w_b[order]
            occ = np.zeros(len(d_b), np.int64)
            if len(d_b):
                is_new = np.ones(len(d_b), bool)
                is_new[1:] = d_b[1:] != d_b[:-1]
                # running count within equal-dst group
                grp_start = np.maximum.accumulate(np.where(is_new, np.arange(len(d_b)), 0))
                occ = np.arange(len(d_b)) - grp_start
                kmax_all = max(kmax_all, int(occ.max()) + 1)
            per_cb[c][b] = (s_b, d_b, w_b, occ)

    kmax = kmax_all
    # static run sizes: max over cores of each (batch, occ) count, 128-aligned
    run_cols = np.zeros((nbatch, kmax), np.int64)
    for b in range(nbatch):
        for k in range(kmax):
            mx = max(int((per_cb[c][b][3] == k).sum()) for c in range(nc))
            run_cols[b, k] = (mx + 127) // 128
    assert run_cols.sum(1).max() * 128 <= NB, (
        f"batch overflow: {run_cols.sum(1).max() * 128} > {NB}")

    # per-batch scatter sub-calls (c0, c1, bufid), each <= CAP_COLS columns
    # (SWDGE calls above ~1024 tokens crash the Q7/device).
    CAP_COLS = 8
    runs = []  # per batch: list of (c0, c1, bufid)
    for b in range(nbatch):
        rb, c0 = [], 0
        for k in range(kmax):
            n = int(run_cols[b, k])
            if n == 0:
                continue
            bufid = 0 if k == 0 else 1
            for s0 in range(0, n, CAP_COLS):
                s1 = min(s0 + CAP_COLS, n)
                rb.append((c0 + s0, c0 + s1, bufid))
            c0 += n
        runs.append(rb)

    gidx = np.zeros((nc, nbatch, 128, NB // 16), np.int16)
    sidx = np.full((nc, nbatch, 128, NB // 16), cfg.trash, np.int16)
    wgt = np.zeros((nc, nbatch, 128, NB // 128), np.float32)
    ti = np.arange(NB)
    rows = (ti % 16)[None, :] + 16 * np.arange(8)[:, None]  # [8, NB]
    cols = ti // 16
    wrow, wcol = ti % 128, ti // 128
    for c in range(nc):
        for b in range(nbatch):
            s_b, d_b, w_b, occ = per_cb[c][b]
            g_lin = np.zeros(NB, np.int16)          # pads gather row 0 of window
            s_lin = np.full(NB, cfg.trash, np.int16)
            w_lin = np.zeros(NB, np.float32)
            c0 = 0
            for k in range(kmax):
                n = int(run_cols[b, k])
                if n == 0:
                    continue
                mk = occ == k
                cnt = int(mk.sum())
                t0 = c0 * 128
                g_lin[t0:t0 + cnt] = (s_b[mk] - b * W).astype(np.int16)
                s_lin[t0:t0 + cnt] = d_b[mk].astype(np.int16)
                w_lin[t0:t0 + cnt] = w_b[mk]
                c0 += n
            for g in range(8):
                gidx[c, b, rows[g], cols] = g_lin
                sidx[c, b, rows[g], cols] = s_lin
            wgt[c, b, wrow, wcol] = w_lin
    return gidx, sidx, wgt, runs


# ----------------------------------------------------------------------
# device program
# ----------------------------------------------------------------------

def build_nc(cfg):
    c = cfg
    nc = bacc.Bacc(None, target_bir_lowering=False, debug=False,
                   num_swdge_queues=2)
    NT, NB, NS, C_, H = c.NT, c.NB, c.NS, c.C, c.H
    nbw = NB // 128          # w / gtile blocks per batch
    L = c.L

    def dram_in(name, shape, dt=F32):
        return nc.declare_dram_parameter(name, shape, dt, isOutput=False)

    xin_t = dram_in("xin_t", [H, NS])
    d_t = dram_in("d_t", [NT, 128, C_])
    p_t = dram_in("p_t", [NT, C_, 128])
    gidx = dram_in("gidx", [c.nbatch, 128, NB // 16], I16)
    sidx = dram_in("sidx", [c.nbatch, 128, NB // 16], I16)
    wgt = dram_in("wgt", [c.nbatch, 128, nbw])
    lin0w = dram_in("lin0w", [H, H])
    lin0b = dram_in("lin0b", [H, 1])
    lin1w = dram_in("lin1w", [H, C_])
    lin1b = dram_in("lin1b", [C_, 1])
    convw = dram_in("convw", [L, H, H])
    cma = dram_in("cma", [C_, C_])
    cmat = dram_in("cmat", [C_, C_])
    i47 = dram_in("i47", [C_, C_])
    ident = dram_in("ident", [128, 128])
    out_t = nc.declare_dram_parameter("out_t", [C_, NS], F32, isOutput=True)

    # internal DRAM
    x_rep = nc.dram_tensor("x_rep", [c.N, H], BF16, addr_space="Shared")
    x_sh = [nc.dram_tensor(f"x_sh{i}", [NS, H], BF16) for i in range(2)]
    x0_sh = nc.dram_tensor("x0_sh", [NS, H], BF16)
    x_T = [nc.dram_tensor(f"x_T{i}", [NT, 128, 128], F32) for i in range(2)]
    x0_T = nc.dram_tensor("x0_T", [NT, 128, 128], F32)
    agg = [nc.dram_tensor(f"agg{i}", [c.nagg, H], BF16) for i in range(2)]
    cen_in = nc.dram_tensor("cen_in", [C_, H], F32)
    cen_out = nc.dram_tensor("cen_out", [C_, H], F32, addr_space="Shared")

    rg = [list(range(c.ncores))]
    betas = [float(np.log(c.theta / (i + 1) + 1.0)) for i in range(L)]

    def tsize(t):
        return min(128, NS - t * 128)

    with tile.TileContext(nc) as tc:
        nc.gpsimd.load_library(library_config.mlp)
        with (
            tc.tile_pool(name="const", bufs=1) as cpool,
            tc.tile_pool(name="sb", bufs=3) as pool,
            tc.tile_pool(name="gt", bufs=2) as gpool,
            tc.tile_pool(name="ps", bufs=3, space="PSUM") as psum,
            tc.tile_pool(name="psacc", bufs=1, space="PSUM") as psacc,
        ):
            # ---- resident constants ----
            lin0w_sb = cpool.tile([H, H], F32)
            nc.sync.dma_start(lin0w_sb[:], lin0w[:, :])
            lin0b_sb = cpool.tile([H, 1], F32)
            nc.sync.dma_start(lin0b_sb[:], lin0b[:, :])
            lin1w_sb = cpool.tile([H, C_], F32)
            nc.sync.dma_start(lin1w_sb[:], lin1w[:, :])
            lin1b_sb = cpool.tile([C_, 1], F32)
            nc.sync.dma_start(lin1b_sb[:], lin1b[:, :])
            convw_sb = cpool.tile([H, L * H], F32)
            for i in range(L):
                nc.sync.dma_start(convw_sb[:, i * H:(i + 1) * H], convw[i])
            cma_sb = cpool.tile([C_, C_], F32)
            nc.sync.dma_start(cma_sb[:], cma[:, :])
            cmat_sb = cpool.tile([C_, C_], F32)
            nc.sync.dma_start(cmat_sb[:], cmat[:, :])
            i47_sb = cpool.tile([C_, C_], F32)
            nc.sync.dma_start(i47_sb[:], i47[:, :])
            ident_sb = cpool.tile([128, 128], F32)
            nc.sync.dma_start(ident_sb[:], ident[:, :])
            zero_sb = cpool.tile([128, 1664], BF16)
            nc.vector.memset(zero_sb[:], 0.0)
            identb_sb = cpool.tile([128, 128], BF16)
            nc.vector.tensor_copy(identb_sb[:], ident_sb[:])

            # ---- lin0: x0 = relu(x @ W0 + b0), write x0_T + x0_sh ----
            for t in range(NT):
                P = tsize(t)
                xi = pool.tile([H, 128], F32, tag="xi")
                nc.sync.dma_start(xi[:, :P], xin_t[:, t * 128:t * 128 + P])
                ps0 = psum.tile([H, 128], F32, tag="b")
                nc.tensor.matmul(ps0[:, :P], lin0w_sb[:], xi[:, :P],
                                 start=True, stop=True)
                x0t = pool.tile([H, 128], F32, tag="x0t")
                nc.scalar.activation(x0t[:, :P], ps0[:, :P],
                                     mybir.ActivationFunctionType.Relu,
                                     bias=lin0b_sb[:, 0:1])
                nc.sync.dma_start(x0_T[t][:, :P], x0t[:, :P])
                x0b = pool.tile([H, 128], BF16, tag="x0b")
                nc.vector.tensor_copy(x0b[:, :P], x0t[:, :P])
                pst = psum.tile([128, 128], BF16, tag="bb")
                nc.tensor.transpose(pst[:P, :], x0b[:, :P], identb_sb[:])
                x0n = pool.tile([128, H], BF16, tag="x0n")
                nc.vector.tensor_copy(x0n[:P, :], pst[:P, :])
                nc.sync.dma_start(x0_sh[t * 128:t * 128 + P, :], x0n[:P, :])

            # ---- layers ----
            for li in range(L):
                beta = betas[li]
                cur_sh = x0_sh if li == 0 else x_sh[li % 2]
                cur_T = x0_T if li == 0 else x_T[li % 2]
                nxt_sh = x_sh[(li + 1) % 2]
                nxt_T = x_T[(li + 1) % 2]

                # - allgather x -
                nc.gpsimd.collective_compute(
                    "AllGather", mybir.AluOpType.bypass, replica_groups=rg,
                    ins=[cur_sh.ap().opt()], outs=[x_rep.ap().opt()],
                )

                # - zero agg buffers -
                for a in range(2):
                    step = 1664
                    for r0 in range(0, c.nagg, step):
                        r1 = min(r0 + step, c.nagg)
                        nc.sync.dma_start(agg[a][r0:r1, :],
                                          zero_sb[:, :r1 - r0])

                # - sparse propagate -
                for b in range(c.nbatch):
                    base = b * c.W
                    wrows = min(c.W + 256, c.N - base)
                    gi = pool.tile([128, NB // 16], I16, tag="gi")
                    nc.sync.dma_start(gi[:], gidx[b])
                    si = pool.tile([128, NB // 16], I16, tag="si")
                    nc.sync.dma_start(si[:], sidx[b])
                    wt = pool.tile([128, nbw], F32, tag="wt")
                    nc.sync.dma_start(wt[:], wgt[b])
                    gtile = gpool.tile([128, nbw, H], BF16, tag="g")
                    # Only the columns actually referenced by scatter runs
                    # need gathering; trailing batch columns are pure pads.
                    used = max(c1 for (_, c1, _) in RUNS[b])
                    # SWDGE calls cap at 1024 tokens (8 cols) - larger
                    # calls wedge the device.
                    for g0 in range(0, used, 8):
                        g1 = min(g0 + 8, nbw)
                        ntok = (g1 - g0) * 128
                        nc.gpsimd.dma_gather(
                            gtile[:, g0:g1, :], x_rep[base:base + wrows, :],
                            gi[:, g0 * 8:g1 * 8],
                            num_idxs=ntok, num_idxs_reg=ntok, elem_size=H,
                            queue_num=1,
                        )
                    for j in range(used):
                        nc.scalar.activation(
                            gtile[:, j, :], gtile[:, j, :],
                            mybir.ActivationFunctionType.Copy,
                            scale=wt[:, j:j + 1],
                        )
                    for (c0, c1, bufid) in RUNS[b]:
                        ntok = (c1 - c0) * 128
                        nc.gpsimd.dma_scatter_add(
                            agg[bufid][:, :], gtile[:, c0:c1, :],
                            si[:, c0 * 8:c1 * 8],
                            num_idxs=ntok, num_idxs_reg=ntok, elem_size=H,
                            queue_num=1,
                        )

                # - pass A: centers partial -
                psA = psacc.tile([C_, H], F32, tag="cen")
                for t in range(NT):
                    P = tsize(t)
                    xa = pool.tile([128, H], BF16, tag="xa")
                    nc.sync.dma_start(xa[:P, :], cur_sh[t * 128:t * 128 + P, :])
                    x0a = pool.tile([128, H], BF16, tag="x0a")
                    nc.sync.dma_start(x0a[:P, :], x0_sh[t * 128:t * 128 + P, :])
                    s = pool.tile([128, H], F32, tag="s")
                    nc.vector.tensor_scalar(s[:P, :], x0a[:P, :], 0.1, None,
                                            mybir.AluOpType.mult)
                    nc.vector.tensor_add(s[:P, :], s[:P, :], xa[:P, :])
                    dt_ = pool.tile([128, C_], F32, tag="dt")
                    nc.sync.dma_start(dt_[:P, :], d_t[t][:P, :])
                    nc.tensor.matmul(psA[:], dt_[:P, :], s[:P, :],
                                     start=(t == 0), stop=(t == NT - 1))
                cenp = pool.tile([C_, H], F32, tag="cenp")
                nc.vector.tensor_copy(cenp[:], psA[:])
                nc.sync.dma_start(cen_in[:, :], cenp[:])
                nc.gpsimd.collective_compute(
                    "AllReduce", mybir.AluOpType.add, replica_groups=rg,
                    ins=[cen_in.ap().opt()], outs=[cen_out.ap().opt()],
                )
                cen = pool.tile([C_, H], F32, tag="cen_sb")
                nc.sync.dma_start(cen[:], cen_out[:, :])

                # - r_cls from centers (Gram trick) -
                pst = psum.tile([128, C_], F32, tag="b")
                nc.tensor.transpose(pst[:, :], cen[:], ident_sb[:C_, :C_])
                cT = pool.tile([128, C_], F32, tag="cT")
                nc.vector.tensor_copy(cT[:], pst[:, :])
                psg = psum.tile([C_, C_], F32, tag="b")
                nc.tensor.matmul(psg[:], cT[:], cT[:], start=True, stop=True)
                g = pool.tile([C_, C_], F32, tag="gg")
                nc.vector.tensor_copy(g[:], psg[:])
                gd = pool.tile([C_, C_], F32, tag="gd")
                nc.vector.tensor_mul(gd[:], g[:], i47_sb[:])
                n2 = pool.tile([C_, 1], F32, tag="n2")
                nc.vector.reduce_sum(n2[:], gd[:], AxisListType.X)
                t1 = pool.tile([C_, C_], F32, tag="t1")
                nc.vector.tensor_scalar(t1[:], g[:], -1.0, n2[:, 0:1],
                                        mybir.AluOpType.mult,
                                        mybir.AluOpType.add)
                ps1 = psum.tile([C_, C_], F32, tag="b")
                nc.tensor.transpose(ps1[:], t1[:], ident_sb[:C_, :C_])
                nrm = pool.tile([C_, C_], F32, tag="nrm")
                nc.vector.tensor_add(nrm[:], t1[:], ps1[:])
                nc.vector.tensor_relu(nrm[:], nrm[:])
                nc.vector.tensor_add(nrm[:], nrm[:], i47_sb[:])
                rn = pool.tile([C_, C_], F32, tag="rn")
                nc.scalar.sqrt(rn[:], nrm[:])
                inv = pool.tile([C_, C_], F32, tag="inv")
                nc.vector.reciprocal(inv[:], rn[:])
                amat = pool.tile([C_, C_], F32, tag="amat")
                nc.vector.tensor_mul(amat[:], cma_sb[:], inv[:])
                atm = pool.tile([C_, C_], F32, tag="atm")
                nc.vector.tensor_mul(atm[:], cmat_sb[:], inv[:])
                rs = pool.tile([C_, 1], F32, tag="rs")
                nc.vector.reduce_sum(rs[:], amat[:], AxisListType.X)
                psm = psum.tile([C_, H], F32, tag="b")
                nc.tensor.matmul(psm[:], atm[:], cen[:], start=True, stop=True)
                rcls = pool.tile([C_, H], F32, tag="rcls")
                nc.vector.tensor_scalar(rcls[:], cen[:], rs[:, 0:1], None,
                                        mybir.AluOpType.mult)
                nc.vector.tensor_sub(rcls[:], rcls[:], psm[:])

                # - pass B -
                for t in range(NT):
                    P = tsize(t)
                    pt = pool.tile([C_, 128], F32, tag="pt")
                    nc.sync.dma_start(pt[:], p_t[t])
                    ps1b = psum.tile([H, 128], F32, tag="b")
                    nc.tensor.matmul(ps1b[:, :P], rcls[:], pt[:, :P],
                                     start=True, stop=True)
                    aA = pool.tile([128, H], BF16, tag="aA")
                    nc.sync.dma_start(aA[:P, :], agg[0][t * 128:t * 128 + P, :])
                    aB = pool.tile([128, H], BF16, tag="aB")
                    nc.sync.dma_start(aB[:P, :], agg[1][t * 128:t * 128 + P, :])
                    aS = pool.tile([128, H], F32, tag="aS")
                    nc.vector.tensor_add(aS[:P, :], aA[:P, :], aB[:P, :])
                    psT = psum.tile([H, 128], F32, tag="b")
                    nc.tensor.transpose(psT[:, :P], aS[:P, :],
                                        ident_sb[:P, :P])
                    xt = pool.tile([H, 128], F32, tag="xt")
                    nc.sync.dma_start(xt[:, :P], cur_T[t][:, :P])
                    x0t2 = pool.tile([H, 128], F32, tag="x0t2")
                    nc.sync.dma_start(x0t2[:, :P], x0_T[t][:, :P])
                    u = pool.tile([H, 128], F32, tag="u")
                    # u = 0.45*(x + aggT + r) + 0.1*x0
                    nc.vector.tensor_add(u[:, :P], xt[:, :P], psT[:, :P])
                    nc.vector.tensor_add(u[:, :P], u[:, :P], ps1b[:, :P])
                    nc.vector.tensor_scalar(u[:, :P], u[:, :P], 0.45, None,
                                            mybir.AluOpType.mult)
                    ux = pool.tile([H, 128], F32, tag="ux")
                    nc.vector.tensor_scalar(ux[:, :P], x0t2[:, :P], 0.1, None,
                                            mybir.AluOpType.mult)
                    nc.vector.tensor_add(u[:, :P], u[:, :P], ux[:, :P])
                    ps2 = psum.tile([H, 128], F32, tag="b")
                    nc.tensor.matmul(ps2[:, :P],
                                     convw_sb[:, li * H:(li + 1) * H],
                                     u[:, :P], start=True, stop=True)
                    o1 = pool.tile([H, 128], F32, tag="o1")
                    nc.vector.tensor_scalar(o1[:, :P], ps2[:, :P], beta, None,
                                            mybir.AluOpType.mult)
                    nc.vector.tensor_scalar(u[:, :P], u[:, :P], 1.0 - beta,
                                            None, mybir.AluOpType.mult)
                    nc.vector.tensor_add(u[:, :P], u[:, :P], o1[:, :P])
                    xn = pool.tile([H, 128], F32, tag="xn")
                    nc.scalar.activation(xn[:, :P], u[:, :P],
                                         mybir.ActivationFunctionType.Relu)
                    nc.sync.dma_start(nxt_T[t][:, :P], xn[:, :P])
                    xnb = pool.tile([H, 128], BF16, tag="xnb")
                    nc.vector.tensor_copy(xnb[:, :P], xn[:, :P])
                    psn = psum.tile([128, 128], BF16, tag="bb")
                    nc.tensor.transpose(psn[:P, :], xnb[:, :P], identb_sb[:])
                    xnn = pool.tile([128, H], BF16, tag="xnn")
                    nc.vector.tensor_copy(xnn[:P, :], psn[:P, :])
                    nc.sync.dma_start(nxt_sh[t * 128:t * 128 + P, :], xnn[:P, :])

            # ---- lin1 ----
            fin_T = x_T[L % 2]
            for t in range(NT):
                P = tsize(t)
                xt = pool.tile([H, 128], F32, tag="fxt")
                nc.sync.dma_start(xt[:, :P], fin_T[t][:, :P])
                psf = psum.tile([C_, 128], F32, tag="b")
                nc.tensor.matmul(psf[:, :P], lin1w_sb[:], xt[:, :P],
                                 start=True, stop=True)
                ot = pool.tile([C_, 128], F32, tag="ot")
                nc.vector.tensor_scalar(ot[:, :P], psf[:, :P],
                                        lin1b_sb[:, 0:1], None,
                                        mybir.AluOpType.add)
                nc.sync.dma_start(out_t[:, t * 128:t * 128 + P], ot[:, :P])

    nc.compile()
    return nc


# RUNS is read by build_nc (static run layout shared across cores)
RUNS = None


# ----------------------------------------------------------------------
# host wrapper
# ----------------------------------------------------------------------

def _prep_inputs(cfg, inputs):
    c = cfg
    x = np.asarray(inputs["x"], np.float32)
    label = np.asarray(inputs["label"], np.int64)
    p = np.asarray(inputs["p"], np.float32)
    cm = np.asarray(inputs["cm"], np.float32)
    lin0_w = np.asarray(inputs["lin0_w"], np.float32)
    lin0_b = np.asarray(inputs["lin0_b"], np.float32)
    lin1_w = np.asarray(inputs["lin1_w"], np.float32)
    lin1_b = np.asarray(inputs["lin1_b"], np.float32)
    conv_w = np.asarray(inputs["conv_w"], np.float32)

    gidx, sidx, wgt, runs = _prep_edges(cfg, inputs["edge_index"],
                                        inputs["edge_weight"])

    cnt = np.bincount(label, minlength=c.C).astype(np.float32)
    cnt = np.maximum(cnt, 1.0)
    NTP = c.NT * 128
    cma = cm[:, 0, :]
    i47 = np.eye(c.C, dtype=np.float32)
    ident = np.eye(128, dtype=np.float32)

    in_maps = []
    for ci in range(c.ncores):
        r0 = ci * c.NS
        xs = x[r0:r0 + c.NS]                      # [NS, H]
        lab = label[r0:r0 + c.NS]
        ps = p[r0:r0 + c.NS]                      # [NS, C]
        d_t = np.zeros((NTP, c.C), np.float32)
        d_t[np.arange(c.NS), lab] = 1.0 / cnt[lab]
        p_pad = np.zeros((NTP, c.C), np.float32)
        p_pad[:c.NS] = ps
        in_maps.append({
            "xin_t": np.ascontiguousarray(xs.T),
            "d_t": np.ascontiguousarray(d_t.reshape(c.NT, 128, c.C)),
            "p_t": np.ascontiguousarray(
                p_pad.reshape(c.NT, 128, c.C).transpose(0, 2, 1)),
            "gidx": gidx[ci], "sidx": sidx[ci], "wgt": wgt[ci],
            "lin0w": lin0_w, "lin0b": lin0_b.reshape(-1, 1),
            "lin1w": lin1_w, "lin1b": lin1_b.reshape(-1, 1),
            "convw": conv_w, "cma": cma,
            "cmat": np.ascontiguousarray(cma.T),
            "i47": i47, "ident": ident,
        })
    return in_maps, runs


_BUILT = {}


def kernel(**inputs):
    cfg = DEF
    global RUNS
    in_maps, runs = _prep_inputs(cfg, inputs)
    key = "default"
    if key not in _BUILT:
        RUNS = runs
        _BUILT[key] = build_nc(cfg)
    nc = _BUILT[key]
    res = bass_utils.run_bass_kernel_spmd(nc, in_maps,
                                          core_ids=list(range(cfg.ncores)))
    outs = [res.results[ci]["out_t"].T for ci in range(cfg.ncores)]
    return np.ascontiguousarray(np.concatenate(outs, 0))



# revision 30
# speedup vs baseline: 1.1679x; 1.1679x over previous
"""GCN2 (nn_GCN2_42331197669873) Bass kernel for 8 TRN2 NeuronCores.

Strategy: graph/data parallel. Nodes sharded row-wise across 8 cores
(12500 each). Per layer:
  - AllGather the node features into a full replica in each core's HBM.
    The class-center partial reduction (pass A) is emitted BEFORE the
    AllGather so its compute overlaps the collective.
  - Sparse propagate: edges bucketed by destination core; within a core,
    edges are assigned to 14 fixed source-windows (7143 rows each) so the
    SWDGE dma_gather can address rows with int16 indices relative to a
    compile-time window base shared by all cores (SPMD).  Gather calls are
    spread round-robin over all 4 SWDGE queues - each queue's descriptor
    generation runs on its own pair of GPSIMD Q7 cores, so 4 queues give
    ~3x SWDGE throughput (microbenchmarked: 8.7 -> 2.9 ns/token).
  - Gathered rows are scaled by edge weights with bulk DVE multiplies
    (per-token weight broadcast along the feature axis) and scatter-added
    into per-(dst-quarter) HBM aggregation buffers, quarter q on SWDGE
    queue q, so the 4 quarters' descriptor generation also runs in
    parallel.  Duplicate dst indices inside one scatter call lose adds on
    HW, so tokens in each (batch, quarter) are grouped into
    occurrence-runs; run k goes to buffer slot (min(k,3), batch parity) -
    6 buffers per quarter - so the WAW chains between same-buffer calls
    are short and off the critical path.  Scatters of batch b are emitted
    after the gathers of batch b+1 so queue-parallel gather desc-gen is
    never head-of-line blocked by waiting scatters.
  - Class-center branch: centers = sum_cores (D_shard^T @ x) + 0.1 * cen0
    with cen0 = D^T @ x0 precomputed once during lin0, tiny [47,128]
    AllReduce (overlapped with the gather phase), then a Gram-matrix
    formulation that never materializes the [47,47,128] pair tensor.
  - Dense phases (lin0 / pass A / pass B / lin1) stream node tiles in
    13-tile chunks with one DMA per stream per chunk - per-tile DMAs made
    the Sync engine's ~0.7us/issue the phase bottleneck.

kernel(**inputs) takes the FULL unsharded inputs and returns the FULL
[100000, 47] output; sharding + all preprocessing happens on host inside.
"""

import math
import ml_dtypes
import numpy as np

from concourse import bass, bacc, tile, mybir, bass_utils
from concourse import library_config
from concourse.mybir import AxisListType
import concourse.tile_sem_assignment as _tsa
from concourse import bass_isa as _bisa

# Tile round-robins Pool-engine DMAs over all DMASW sem lanes ignoring
# queue_num; mixing SWDGE queues on one lane breaks its in-order-completion
# assumption (sim: "sem locked to SWDGE queue"). Segregate lanes by queue:
# queue q -> lanes [2q, 2q+2).
_orig_assign_tick = _tsa.TileClockTick._assign_tick

def _assign_tick_qsplit(self, inst):
    if (isinstance(inst, _tsa.DMAInst)
            and inst.engine == mybir.EngineType.Pool
            and not isinstance(inst, _bisa.UserSyncedRemoteDMADescs)
            and self.swdge_sem_count >= 8):
        qn = getattr(inst, "queue_num", 0) or 0
        if not hasattr(self, "_qrr"):
            self._qrr = {}
        r = self._qrr.get(qn, 0)
        self._qrr[qn] = r + 1
        self.next_sw_dma_idx = (qn % 4) * 2 + r % 2
    return _orig_assign_tick(self, inst)

_tsa.TileClockTick._assign_tick = _assign_tick_qsplit

F32 = mybir.dt.float32
BF16 = mybir.dt.bfloat16
I16 = mybir.dt.int16

NQ = 4          # SWDGE queues
NQUART = 4      # dst quarters (1:1 with queues for scatter)
NSLOT = 6       # agg buffer slots per quarter: (k0,p0),(k0,p1),(k1,p0),(k1,p1),k2,k3+


def _slot_of(k, b):
    ks = min(k, 3)
    if ks <= 1:
        return ks * 2 + (b % 2)
    return 2 + ks


class Cfg:
    def __init__(self, N=100000, E=800000, C=47, H=128, ncores=8, nbatch=14,
                 L=4, alpha=0.1, theta=0.5, rsl=0.5):
        self.N, self.E, self.C, self.H = N, E, C, H
        self.ncores = ncores
        self.NS = N // ncores                 # nodes per core
        self.NT = (self.NS + 127) // 128      # node tiles per core
        self.NTP = self.NT * 128              # padded rows per core
        self.Npad = self.NTP * ncores         # padded global rows
        self.nbatch = nbatch                  # src windows
        # window width over PADDED global rows (int16-safe rel idx)
        self.W = (self.Npad + nbatch - 1) // nbatch
        assert self.W + 256 < 32768
        self.NB = None                        # tokens per batch (set by prep)
        self.L, self.alpha, self.theta, self.rsl = L, alpha, theta, rsl
        # dst quarters, tile-aligned: 25/25/25/23 tiles
        qt = [25, 25, 25, self.NT - 75]
        self.q_tiles = qt
        self.q_base_tile = [0, 25, 50, 75]
        self.q_rows = [t * 128 for t in qt]          # rows per quarter
        self.q_base = [t * 128 for t in self.q_base_tile]
        # dense-phase chunks (tiles), quarter-aligned, <= CK tiles each
        self.CK = 7
        self.chunks = []
        for q in range(NQUART):
            t0 = self.q_base_tile[q]
            nt = qt[q]
            nch = (nt + self.CK - 1) // self.CK
            base, rem = divmod(nt, nch)
            off = t0
            for j in range(nch):
                sz = base + (1 if j < rem else 0)
                self.chunks.append((off, off + sz, q))
                off += sz


DEF = Cfg()


# ----------------------------------------------------------------------
# host-side edge preprocessing
# ----------------------------------------------------------------------

def _prep_edges(cfg, edge_index, edge_weight):
    """Per core: batch/window token layout, dst-quartered with occurrence runs.

    Token layout within a batch: quarters in order, within a quarter the
    occurrence-run classes in order, each (quarter, k) run padded to a
    128-token column boundary.  Shared (SPMD) static layout = max over cores.

    Returns gidx [nc, nbatch, 128, NB//16] i16 (window-relative src),
    sidx (same shape; quarter-buffer-relative dst), wgt [nc, nbatch, 128,
    NB//128] f32, runs: per batch a list of (c0, c1, quarter, slot), used:
    per batch the number of used columns, NB.
    """
    src = np.asarray(edge_index[0], np.int64)
    dst = np.asarray(edge_index[1], np.int64)
    w = np.asarray(edge_weight, np.float32)
    nc, NS, nbatch, W = cfg.ncores, cfg.NS, cfg.nbatch, cfg.W

    # src in PADDED global coordinates (x_rep rows are NTP per core)
    psrc = (src // NS) * cfg.NTP + src % NS

    per_cbq = [[[None] * NQUART for _ in range(nbatch)] for _ in range(nc)]
    kmax = 1
    for c in range(nc):
        m = (dst >= c * NS) & (dst < (c + 1) * NS)
        s_c, d_c, w_c = psrc[m], dst[m] - c * NS, w[m]
        b_c = s_c // W
        q_c = np.minimum(d_c // 3200, NQUART - 1)
        for b in range(nbatch):
            for q in range(NQUART):
                mb = (b_c == b) & (q_c == q)
                s_b, d_b, w_b = s_c[mb], d_c[mb], w_c[mb]
                order = np.argsort(d_b, kind="stable")
                s_b, d_b, w_b = s_b[order], d_b[order], w_b[order]
                occ = np.zeros(len(d_b), np.int64)
                if len(d_b):
                    is_new = np.ones(len(d_b), bool)
                    is_new[1:] = d_b[1:] != d_b[:-1]
                    grp_start = np.maximum.accumulate(
                        np.where(is_new, np.arange(len(d_b)), 0))
                    occ = np.arange(len(d_b)) - grp_start
                    kmax = max(kmax, int(occ.max()) + 1)
                per_cbq[c][b][q] = (s_b, d_b, w_b, occ)

    run_cols = np.zeros((nbatch, NQUART, kmax), np.int64)
    for b in range(nbatch):
        for q in range(NQUART):
            for k in range(kmax):
                mx = max(int((per_cbq[c][b][q][3] == k).sum())
                         for c in range(nc))
                run_cols[b, q, k] = (mx + 127) // 128

    max_cols = int(run_cols.sum(axis=(1, 2)).max())
    NB = ((max_cols + 7) // 8) * 8 * 128   # pad to 8-col gather call units
    cfg.NB = NB

    # per-batch scatter sub-calls, each <= CAP_COLS columns (SWDGE calls
    # above ~1024 tokens crash the Q7/device). Emission interleaves
    # (cap-chunk-major, then quarter) so same-buffer calls are spaced.
    CAP_COLS = 8
    runs = []
    used_cols = []
    col0 = np.zeros((nbatch, NQUART, kmax), np.int64)
    for b in range(nbatch):
        c0 = 0
        for q in range(NQUART):
            for k in range(kmax):
                n = int(run_cols[b, q, k])
                col0[b, q, k] = c0
                c0 += n
        used_cols.append(c0)
        rb = []
        for k in range(kmax):
            for s in range(0, int(run_cols[b, :, k].max()), CAP_COLS):
                for q in range(NQUART):
                    n = int(run_cols[b, q, k])
                    if s >= n:
                        continue
                    s1 = min(s + CAP_COLS, n)
                    rb.append((int(col0[b, q, k] + s),
                               int(col0[b, q, k] + s1), q, _slot_of(k, b)))
        runs.append(rb)

    gidx = np.zeros((nc, nbatch, 128, NB // 16), np.int16)
    sidx = np.zeros((nc, nbatch, 128, NB // 16), np.int16)
    wgt = np.zeros((nc, nbatch, 128, NB // 128), np.float32)
    ti = np.arange(NB)
    rows = (ti % 16)[None, :] + 16 * np.arange(8)[:, None]  # [8, NB]
    cols = ti // 16
    wrow, wcol = ti % 128, ti // 128
    for c in range(nc):
        for b in range(nbatch):
            g_lin = np.zeros(NB, np.int16)       # pads gather row 0 of window
            s_lin = np.zeros(NB, np.int16)
            w_lin = np.zeros(NB, np.float32)
            for q in range(NQUART):
                for k in range(kmax):
                    n = int(run_cols[b, q, k])
                    if n == 0:
                        continue
                    t0 = int(col0[b, q, k]) * 128
                    s_lin[t0:t0 + n * 128] = cfg.q_rows[q]   # trash default
                    s_b, d_b, w_b, occ = per_cbq[c][b][q]
                    mk = occ == k
                    cnt = int(mk.sum())
                    g_lin[t0:t0 + cnt] = (s_b[mk] - b * W).astype(np.int16)
                    s_lin[t0:t0 + cnt] = (d_b[mk] - cfg.q_base[q]).astype(
                        np.int16)
                    w_lin[t0:t0 + cnt] = w_b[mk]
            for g in range(8):
                gidx[c, b, rows[g], cols] = g_lin
                sidx[c, b, rows[g], cols] = s_lin
            wgt[c, b, wrow, wcol] = w_lin
    return gidx, sidx, wgt, runs, used_cols, NB


# ----------------------------------------------------------------------
# device program
# ----------------------------------------------------------------------

def build_nc(cfg):
    c = cfg
    nc = bacc.Bacc(None, target_bir_lowering=False, debug=False,
                   num_swdge_queues=NQ)
    NT, NB, NS, C_, H = c.NT, c.NB, c.NS, c.C, c.H
    nbw = NB // 128          # w / gtile blocks per batch
    L = c.L

    def dram_in(name, shape, dt=F32):
        return nc.declare_dram_parameter(name, shape, dt, isOutput=False)

    xin_t = dram_in("xin_t", [H, NS])
    d_t = dram_in("d_t", [NT, 128, C_], BF16)
    p_t = dram_in("p_t", [NT, C_, 128])
    gidx = dram_in("gidx", [c.nbatch, 128, NB // 16], I16)
    sidx = dram_in("sidx", [c.nbatch, 128, NB // 16], I16)
    wgt = dram_in("wgt", [c.nbatch, 128, nbw])
    lin0w = dram_in("lin0w", [H, H])
    lin0b = dram_in("lin0b", [H, 1])
    lin1w = dram_in("lin1w", [H, C_])
    lin1b = dram_in("lin1b", [C_, 1])
    convw = dram_in("convw", [L, H, H])
    cma = dram_in("cma", [C_, C_])
    cmat = dram_in("cmat", [C_, C_])
    i47 = dram_in("i47", [C_, C_])
    ident = dram_in("ident", [128, 128])
    invcnt = dram_in("invcnt", [C_, 1])
    out_t = nc.declare_dram_parameter("out_t", [C_, c.NTP], F32,
                                      isOutput=True)
    if DEBUG_DUMP:
        dbg_cen = nc.declare_dram_parameter("dbg_cen", [L, C_, H], F32,
                                            isOutput=True)
        dbg_agg = nc.declare_dram_parameter("dbg_agg", [NSLOT, 3328, H], F32,
                                            isOutput=True)
        dbg_x1 = nc.declare_dram_parameter("dbg_x1", [NT, 128, 128], F32,
                                           isOutput=True)

    # internal DRAM (node-major tensors padded to whole tiles)
    x_rep = nc.dram_tensor("x_rep", [c.Npad, H], BF16, addr_space="Shared")
    x_sh = [nc.dram_tensor(f"x_sh{i}", [c.NTP, H], BF16) for i in range(2)]
    x0_sh = nc.dram_tensor("x0_sh", [c.NTP, H], BF16)
    x_T = [nc.dram_tensor(f"x_T{i}", [NT, 128, 128], F32) for i in range(2)]
    x0_T = nc.dram_tensor("x0_T", [NT, 128, 128], F32)
    # per-quarter aggregation buffer slots, each with a trailing 128-row
    # trash pad block
    agg = [[nc.dram_tensor(f"agg{q}_{i}", [c.q_rows[q] + 128, H], BF16)
            for i in range(NSLOT)] for q in range(NQUART)]
    cen_in = nc.dram_tensor("cen_in", [C_, H], F32)
    cen_out = nc.dram_tensor("cen_out", [C_, H], F32, addr_space="Shared")

    rg = [list(range(c.ncores))]
    betas = [float(np.log(c.theta / (i + 1) + 1.0)) for i in range(L)]

    def tsize(t):
        return min(128, NS - t * 128)

    def rows_of(t0, t1):
        # node-major DRAM rows [t0*128, t1*128) viewed as [128, ct, H]
        return (t0 * 128, t1 * 128)

    with tile.TileContext(nc) as tc:
        nc.gpsimd.load_library(library_config.mlp)
        with (
            tc.tile_pool(name="const", bufs=1) as cpool,
            tc.tile_pool(name="sb", bufs=3) as pool,
            tc.tile_pool(name="ck", bufs=2) as ckpool,
            tc.tile_pool(name="gt", bufs=2) as gpool,
            tc.tile_pool(name="ps", bufs=3, space="PSUM") as psum,
            tc.tile_pool(name="psacc", bufs=1, space="PSUM") as psacc,
        ):
            # ---- resident constants ----
            lin0w_sb = cpool.tile([H, H], F32)
            nc.sync.dma_start(lin0w_sb[:], lin0w[:, :])
            lin0b_sb = cpool.tile([H, 1], F32)
            nc.sync.dma_start(lin0b_sb[:], lin0b[:, :])
            lin1w_sb = cpool.tile([H, C_], F32)
            nc.sync.dma_start(lin1w_sb[:], lin1w[:, :])
            lin1b_sb = cpool.tile([C_, 1], F32)
            nc.sync.dma_start(lin1b_sb[:], lin1b[:, :])
            convw_sb = cpool.tile([H, L * H], F32)
            for i in range(L):
                nc.sync.dma_start(convw_sb[:, i * H:(i + 1) * H], convw[i])
            cma_sb = cpool.tile([C_, C_], F32)
            nc.sync.dma_start(cma_sb[:], cma[:, :])
            cmat_sb = cpool.tile([C_, C_], F32)
            nc.sync.dma_start(cmat_sb[:], cmat[:, :])
            i47_sb = cpool.tile([C_, C_], F32)
            nc.sync.dma_start(i47_sb[:], i47[:, :])
            ident_sb = cpool.tile([128, 128], F32)
            nc.sync.dma_start(ident_sb[:], ident[:, :])
            invcnt_sb = cpool.tile([C_, 1], F32)
            nc.sync.dma_start(invcnt_sb[:], invcnt[:, :])
            zero_sb = cpool.tile([128, 1664], BF16)
            nc.vector.memset(zero_sb[:], 0.0)
            identb_sb = cpool.tile([128, 128], BF16)
            nc.vector.tensor_copy(identb_sb[:], ident_sb[:])
            cen0_sb = cpool.tile([C_, H], F32)   # D^T @ x0 (layer-invariant)

            # ---- lin0: x0 = relu(x @ W0 + b0) (chunked), write x0_T +
            #      x0_sh; also accumulate cen0 = D^T @ x0 ----
            psc0 = psacc.tile([C_, H], F32, tag="cen")
            for ci_, (t0, t1, _) in enumerate(c.chunks):
                ct = t1 - t0
                r0, r1 = rows_of(t0, t1)
                xi = ckpool.tile([H, c.CK * 128], F32, tag="xi")
                rr1 = min(r1, NS)
                nc.sync.dma_start(xi[:, :rr1 - r0], xin_t[:, r0:rr1])
                dt_ = ckpool.tile([128, c.CK, C_], BF16, tag="dt")
                nc.sync.dma_start(dt_[:, :ct, :], d_t[t0:t1].rearrange(
                    "t p c -> p t c"))
                x0ck = ckpool.tile([128, c.CK, 128], F32, tag="x0ck")
                x0nck = ckpool.tile([128, c.CK, H], BF16, tag="x0nck")
                for i in range(ct):
                    t = t0 + i
                    P = tsize(t)
                    ps0 = psum.tile([H, 128], F32, tag="b")
                    nc.tensor.matmul(ps0[:, :P], lin0w_sb[:],
                                     xi[:, i * 128:i * 128 + P],
                                     start=True, stop=True)
                    nc.scalar.activation(x0ck[:, i, :P], ps0[:, :P],
                                         mybir.ActivationFunctionType.Relu,
                                         bias=lin0b_sb[:, 0:1])
                    x0b = pool.tile([H, 128], BF16, tag="x0b")
                    nc.vector.tensor_copy(x0b[:, :P], x0ck[:, i, :P])
                    pst = psum.tile([128, 128], BF16, tag="bb")
                    nc.tensor.transpose(pst[:P, :], x0b[:, :P], identb_sb[:])
                    nc.vector.tensor_copy(x0nck[:P, i, :], pst[:P, :])
                    nc.tensor.matmul(psc0[:], dt_[:P, i, :], x0nck[:P, i, :],
                                     start=(t == 0), stop=(t == NT - 1))
                nc.sync.dma_start(x0_T[t0:t1].rearrange("t p n -> p t n"),
                                  x0ck[:, :ct, :])
                nc.sync.dma_start(
                    x0_sh[r0:r1, :].rearrange("(t p) h -> p t h", p=128),
                    x0nck[:, :ct, :])
            nc.vector.tensor_copy(cen0_sb[:], psc0[:])

            # ---- layers ----
            for li in range(L):
                beta = betas[li]
                cur_sh = x0_sh if li == 0 else x_sh[li % 2]
                cur_T = x0_T if li == 0 else x_T[li % 2]
                nxt_sh = x_sh[(li + 1) % 2]
                nxt_T = x_T[(li + 1) % 2]

                # - pass A: centers partial (chunked; emitted before the
                #   AllGather so its work overlaps the collective) -
                psA = psacc.tile([C_, H], F32, tag="cen")
                for (t0, t1, _) in c.chunks:
                    ct = t1 - t0
                    r0, r1 = rows_of(t0, t1)
                    xa = ckpool.tile([128, c.CK, H], BF16, tag="xa")
                    nc.sync.dma_start(
                        xa[:, :ct, :],
                        cur_sh[r0:r1, :].rearrange("(t p) h -> p t h", p=128))
                    dt_ = ckpool.tile([128, c.CK, C_], BF16, tag="dt")
                    nc.sync.dma_start(dt_[:, :ct, :], d_t[t0:t1].rearrange(
                        "t p c -> p t c"))
                    for i in range(ct):
                        t = t0 + i
                        P = tsize(t)
                        nc.tensor.matmul(psA[:], dt_[:P, i, :], xa[:P, i, :],
                                         start=(t == 0), stop=(t == NT - 1))
                cenp = pool.tile([C_, H], F32, tag="cenp")
                nc.vector.scalar_tensor_tensor(
                    cenp[:], cen0_sb[:], 0.1, psA[:],
                    op0=mybir.AluOpType.mult, op1=mybir.AluOpType.add)
                nc.sync.dma_start(cen_in[:, :], cenp[:])

                # - allgather x -
                nc.gpsimd.collective_compute(
                    "AllGather", mybir.AluOpType.bypass, replica_groups=rg,
                    ins=[cur_sh.ap().opt()], outs=[x_rep.ap().opt()],
                )
                # - centers allreduce (overlaps the gather phase) -
                nc.gpsimd.collective_compute(
                    "AllReduce", mybir.AluOpType.add, replica_groups=rg,
                    ins=[cen_in.ap().opt()], outs=[cen_out.ap().opt()],
                )

                # - zero agg buffers (overlaps the collectives) -
                for q in range(NQUART):
                    for a in range(NSLOT):
                        nrow = c.q_rows[q] + 128
                        step = 1664
                        for r0 in range(0, nrow, step):
                            r1 = min(r0 + step, nrow)
                            nc.sync.dma_start(agg[q][a][r0:r1, :],
                                              zero_sb[:, :r1 - r0])

                # - sparse propagate; scatters of batch b follow the gathers
                #   of batch b+1 so waiting scatters never head-of-line
                #   block queue-parallel gather desc-gen -
                gts = {}
                sis = {}
                for b in range(c.nbatch + 1):
                    if b < c.nbatch:
                        base = b * c.W
                        wrows = min(c.W + 256, c.Npad - base)
                        used = USED[b]
                        gi = pool.tile([128, NB // 16], I16, tag="gi")
                        nc.sync.dma_start(gi[:], gidx[b])
                        si = pool.tile([128, NB // 16], I16, tag="si")
                        nc.sync.dma_start(si[:], sidx[b])
                        wt = pool.tile([128, nbw], F32, tag="wt")
                        nc.sync.dma_start(wt[:], wgt[b])
                        gtile = gpool.tile([128, nbw, H], BF16, tag="g")
                        gts[b] = gtile
                        sis[b] = si
                        for gci, g0 in enumerate(range(0, used, 8)):
                            g1 = min(g0 + 8, nbw)
                            ntok = (g1 - g0) * 128
                            nc.gpsimd.dma_gather(
                                gtile[:, g0:g1, :],
                                x_rep[base:base + wrows, :],
                                gi[:, g0 * 8:g1 * 8],
                                num_idxs=ntok, num_idxs_reg=ntok,
                                elem_size=H, queue_num=(b + gci) % NQ,
                            )
                            wt3 = wt[:, g0:g1].unsqueeze(2).broadcast_to(
                                (128, g1 - g0, H))
                            nc.vector.tensor_mul(gtile[:, g0:g1, :],
                                                 gtile[:, g0:g1, :], wt3)
                    if b >= 1:
                        bp = b - 1
                        gtile_p, si_p = gts.pop(bp), sis.pop(bp)
                        for (c0, c1, q, slot) in RUNS[bp]:
                            ntok = (c1 - c0) * 128
                            nc.gpsimd.dma_scatter_add(
                                agg[q][slot][:, :], gtile_p[:, c0:c1, :],
                                si_p[:, c0 * 8:c1 * 8],
                                num_idxs=ntok, num_idxs_reg=ntok,
                                elem_size=H, queue_num=q,
                            )

                # - r_cls from centers (Gram trick); d_t is an exact one-hot
                #   so the f32 1/cnt scale happens here, AFTER the reduce -
                cenr = pool.tile([C_, H], F32, tag="cenr")
                nc.sync.dma_start(cenr[:], cen_out[:, :])
                cen = pool.tile([C_, H], F32, tag="cen_sb")
                nc.vector.tensor_scalar(cen[:], cenr[:], invcnt_sb[:, 0:1],
                                        None, mybir.AluOpType.mult)
                if DEBUG_DUMP:
                    nc.sync.dma_start(dbg_cen[li], cen[:])
                    if li == 0:
                        for slot in range(NSLOT):
                            for r0 in range(0, 3328, c.CK * 128):
                                da = pool.tile([128, c.CK, H], BF16, tag="dbgda")
                                nc.sync.dma_start(
                                    da[:, :min(c.CK*128, 3328-r0)//128, :],
                                    agg[0][slot][r0:r0 + min(c.CK*128, 3328-r0), :].rearrange(
                                        "(t p) h -> p t h", p=128))
                                daf = pool.tile([128, c.CK, H], F32, tag="dbgdf")
                                nc.vector.tensor_copy(daf[:], da[:])
                                nc.sync.dma_start(
                                    dbg_agg[slot][r0:r0 + 1664, :].rearrange(
                                        "(t p) h -> p t h", p=128), daf[:])
                pst = psum.tile([128, C_], F32, tag="b")
                nc.tensor.transpose(pst[:, :], cen[:], ident_sb[:C_, :C_])
                cT = pool.tile([128, C_], F32, tag="cT")
                nc.vector.tensor_copy(cT[:], pst[:, :])
                psg = psum.tile([C_, C_], F32, tag="b")
                nc.tensor.matmul(psg[:], cT[:], cT[:], start=True, stop=True)
                g = pool.tile([C_, C_], F32, tag="gg")
                nc.vector.tensor_copy(g[:], psg[:])
                gd = pool.tile([C_, C_], F32, tag="gd")
                nc.vector.tensor_mul(gd[:], g[:], i47_sb[:])
                n2 = pool.tile([C_, 1], F32, tag="n2")
                nc.vector.reduce_sum(n2[:], gd[:], AxisListType.X)
                t1_ = pool.tile([C_, C_], F32, tag="t1")
                nc.vector.tensor_scalar(t1_[:], g[:], -1.0, n2[:, 0:1],
                                        mybir.AluOpType.mult,
                                        mybir.AluOpType.add)
                ps1 = psum.tile([C_, C_], F32, tag="b")
                nc.tensor.transpose(ps1[:], t1_[:], ident_sb[:C_, :C_])
                nrm = pool.tile([C_, C_], F32, tag="nrm")
                nc.vector.tensor_add(nrm[:], t1_[:], ps1[:])
                nc.vector.tensor_relu(nrm[:], nrm[:])
                nc.vector.tensor_add(nrm[:], nrm[:], i47_sb[:])
                rn = pool.tile([C_, C_], F32, tag="rn")
                nc.scalar.sqrt(rn[:], nrm[:])
                inv = pool.tile([C_, C_], F32, tag="inv")
                nc.vector.reciprocal(inv[:], rn[:])
                amat = pool.tile([C_, C_], F32, tag="amat")
                nc.vector.tensor_mul(amat[:], cma_sb[:], inv[:])
                atm = pool.tile([C_, C_], F32, tag="atm")
                nc.vector.tensor_mul(atm[:], cmat_sb[:], inv[:])
                rs = pool.tile([C_, 1], F32, tag="rs")
                nc.vector.reduce_sum(rs[:], amat[:], AxisListType.X)
                psm = psum.tile([C_, H], F32, tag="b")
                nc.tensor.matmul(psm[:], atm[:], cen[:], start=True, stop=True)
                rcls = pool.tile([C_, H], F32, tag="rcls")
                nc.vector.tensor_scalar(rcls[:], cen[:], rs[:, 0:1], None,
                                        mybir.AluOpType.mult)
                nc.vector.tensor_sub(rcls[:], rcls[:], psm[:])

                # - pass B (chunked) -
                for (t0, t1, q) in c.chunks:
                    ct = t1 - t0
                    r0, r1 = rows_of(t0, t1)
                    lr0 = t0 * 128 - c.q_base[q]
                    # sum the quarter's agg slots for this chunk
                    asum = ckpool.tile([128, c.CK, H], F32, tag="asum")
                    for slot in range(NSLOT):
                        ast = pool.tile([128, c.CK, H], BF16, tag="ast")
                        nc.sync.dma_start(
                            ast[:, :ct, :],
                            agg[q][slot][lr0:lr0 + ct * 128, :].rearrange(
                                "(t p) h -> p t h", p=128))
                        if slot == 0:
                            nc.vector.tensor_copy(asum[:, :ct, :],
                                                  ast[:, :ct, :])
                        else:
                            nc.vector.tensor_add(asum[:, :ct, :],
                                                 asum[:, :ct, :],
                                                 ast[:, :ct, :])
                    pt = ckpool.tile([C_, c.CK, 128], F32, tag="pt")
                    nc.sync.dma_start(pt[:, :ct, :], p_t[t0:t1].rearrange(
                        "t c n -> c t n"))
                    xt = ckpool.tile([128, c.CK, 128], F32, tag="xt")
                    nc.sync.dma_start(xt[:, :ct, :], cur_T[t0:t1].rearrange(
                        "t h n -> h t n"))
                    x0t2 = ckpool.tile([128, c.CK, 128], F32, tag="x0t2")
                    nc.sync.dma_start(x0t2[:, :ct, :], x0_T[t0:t1].rearrange(
                        "t h n -> h t n"))
                    xnck = ckpool.tile([128, c.CK, 128], F32, tag="xnck")
                    xnsh = ckpool.tile([128, c.CK, H], BF16, tag="xnsh")
                    for i in range(ct):
                        t = t0 + i
                        P = tsize(t)
                        ps1b = psum.tile([H, 128], F32, tag="b")
                        nc.tensor.matmul(ps1b[:, :P], rcls[:], pt[:, i, :P],
                                         start=True, stop=True)
                        psT = psum.tile([H, 128], F32, tag="b")
                        nc.tensor.transpose(psT[:, :P], asum[:P, i, :],
                                            ident_sb[:P, :P])
                        # u = 0.45*(x + aggT + r) + 0.1*x0
                        s1 = pool.tile([H, 128], F32, tag="s1")
                        nc.vector.tensor_add(s1[:, :P], xt[:, i, :P],
                                             psT[:, :P])
                        nc.vector.tensor_add(s1[:, :P], s1[:, :P],
                                             ps1b[:, :P])
                        ux = pool.tile([H, 128], F32, tag="ux")
                        nc.vector.tensor_scalar(ux[:, :P], x0t2[:, i, :P],
                                                0.1, None,
                                                mybir.AluOpType.mult)
                        u = pool.tile([H, 128], F32, tag="u")
                        nc.vector.scalar_tensor_tensor(
                            u[:, :P], s1[:, :P], 0.45, ux[:, :P],
                            op0=mybir.AluOpType.mult, op1=mybir.AluOpType.add)
                        ps2 = psum.tile([H, 128], F32, tag="b")
                        nc.tensor.matmul(ps2[:, :P],
                                         convw_sb[:, li * H:(li + 1) * H],
                                         u[:, :P], start=True, stop=True)
                        # x_next = relu((1-b) * (u + (b/(1-b))*(u@W)))
                        t3 = pool.tile([H, 128], F32, tag="t3")
                        nc.vector.scalar_tensor_tensor(
                            t3[:, :P], ps2[:, :P], beta / (1.0 - beta),
                            u[:, :P], op0=mybir.AluOpType.mult,
                            op1=mybir.AluOpType.add)
                        nc.scalar.activation(xnck[:, i, :P], t3[:, :P],
                                             mybir.ActivationFunctionType.Relu,
                                             scale=1.0 - beta)
                        xnb = pool.tile([H, 128], BF16, tag="xnb")
                        nc.vector.tensor_copy(xnb[:, :P], xnck[:, i, :P])
                        psn = psum.tile([128, 128], BF16, tag="bb")
                        nc.tensor.transpose(psn[:P, :], xnb[:, :P],
                                            identb_sb[:])
                        nc.vector.tensor_copy(xnsh[:P, i, :], psn[:P, :])
                    nc.sync.dma_start(nxt_T[t0:t1].rearrange(
                        "t h n -> h t n"), xnck[:, :ct, :])
                    nc.sync.dma_start(
                        nxt_sh[r0:r1, :].rearrange("(t p) h -> p t h", p=128),
                        xnsh[:, :ct, :])
                    if DEBUG_DUMP and li == 0:
                        nc.sync.dma_start(dbg_x1[t0:t1].rearrange(
                            "t h n -> h t n"), xnck[:, :ct, :])

            # ---- lin1 (chunked) ----
            fin_T = x_T[L % 2]
            for (t0, t1, _) in c.chunks:
                ct = t1 - t0
                r0, r1 = rows_of(t0, t1)
                xt = ckpool.tile([128, c.CK, 128], F32, tag="fxt")
                nc.sync.dma_start(xt[:, :ct, :], fin_T[t0:t1].rearrange(
                    "t h n -> h t n"))
                otck = ckpool.tile([C_, c.CK, 128], F32, tag="ot")
                for i in range(ct):
                    t = t0 + i
                    P = tsize(t)
                    psf = psum.tile([C_, 128], F32, tag="b")
                    nc.tensor.matmul(psf[:, :P], lin1w_sb[:], xt[:, i, :P],
                                     start=True, stop=True)
                    nc.vector.tensor_scalar(otck[:, i, :P], psf[:, :P],
                                            lin1b_sb[:, 0:1], None,
                                            mybir.AluOpType.add)
                nc.sync.dma_start(out_t[:, r0:r1].rearrange(
                    "c (t n) -> c t n", n=128), otck[:, :ct, :])

    nc.compile()
    return nc


# RUNS / USED are read by build_nc (static layout shared across cores)
RUNS = None
USED = None
DEBUG_DUMP = False


# ----------------------------------------------------------------------
# host wrapper
# ----------------------------------------------------------------------

def _prep_inputs(cfg, inputs):
    c = cfg
    x = np.asarray(inputs["x"], np.float32)
    label = np.asarray(inputs["label"], np.int64)
    p = np.asarray(inputs["p"], np.float32)
    cm = np.asarray(inputs["cm"], np.float32)
    lin0_w = np.asarray(inputs["lin0_w"], np.float32)
    lin0_b = np.asarray(inputs["lin0_b"], np.float32)
    lin1_w = np.asarray(inputs["lin1_w"], np.float32)
    lin1_b = np.asarray(inputs["lin1_b"], np.float32)
    conv_w = np.asarray(inputs["conv_w"], np.float32)

    gidx, sidx, wgt, runs, used, NB = _prep_edges(
        cfg, inputs["edge_index"], inputs["edge_weight"])

    cnt = np.bincount(label, minlength=c.C).astype(np.float32)
    cnt = np.maximum(cnt, 1.0)
    NTP = c.NT * 128
    cma = cm[:, 0, :]
    i47 = np.eye(c.C, dtype=np.float32)
    ident = np.eye(128, dtype=np.float32)

    in_maps = []
    for ci in range(c.ncores):
        r0 = ci * c.NS
        xs = x[r0:r0 + c.NS]                      # [NS, H]
        lab = label[r0:r0 + c.NS]
        ps = p[r0:r0 + c.NS]                      # [NS, C]
        d_t = np.zeros((NTP, c.C), np.float32)
        d_t[np.arange(c.NS), lab] = 1.0
        p_pad = np.zeros((NTP, c.C), np.float32)
        p_pad[:c.NS] = ps
        in_maps.append({
            "xin_t": np.ascontiguousarray(xs.T),
            "d_t": np.ascontiguousarray(
                d_t.reshape(c.NT, 128, c.C)).astype(ml_dtypes.bfloat16),
            "p_t": np.ascontiguousarray(
                p_pad.reshape(c.NT, 128, c.C).transpose(0, 2, 1)),
            "gidx": gidx[ci], "sidx": sidx[ci], "wgt": wgt[ci],
            "lin0w": lin0_w, "lin0b": lin0_b.reshape(-1, 1),
            "lin1w": lin1_w, "lin1b": lin1_b.reshape(-1, 1),
            "convw": conv_w, "cma": cma,
            "cmat": np.ascontiguousarray(cma.T),
            "i47": i47, "ident": ident,
            "invcnt": (1.0 / cnt).reshape(-1, 1).astype(np.float32),
        })
    return in_maps, runs, used


_BUILT = {}


def kernel(**inputs):
    cfg = DEF
    global RUNS, USED
    in_maps, runs, used = _prep_inputs(cfg, inputs)
    key = "default"
    if key not in _BUILT:
        RUNS = runs
        USED = used
        _BUILT[key] = build_nc(cfg)
    nc = _BUILT[key]
    res = bass_utils.run_bass_kernel_spmd(nc, in_maps,
                                          core_ids=list(range(cfg.ncores)))
    outs = [res.results[ci]["out_t"].T[:cfg.NS] for ci in range(cfg.ncores)]
    return np.ascontiguousarray(np.concatenate(outs, 0))


# revision 32
# speedup vs baseline: 1.3622x; 1.1664x over previous
"""GCN2 (nn_GCN2_42331197669873) Bass kernel for 8 TRN2 NeuronCores.

Strategy: graph/data parallel. Nodes sharded row-wise across 8 cores
(12500 each). Per layer:
  - AllGather the node features into a full replica in each core's HBM.
    The class-center partial reduction (pass A) is emitted BEFORE the
    AllGather so its compute overlaps the collective.
  - Sparse propagate: edges bucketed by destination core; within a core,
    edges are assigned to 14 fixed source-windows (7143 rows each) so the
    SWDGE dma_gather can address rows with int16 indices relative to a
    compile-time window base shared by all cores (SPMD).  Gather calls are
    spread round-robin over all 4 SWDGE queues - each queue's descriptor
    generation runs on its own pair of GPSIMD Q7 cores, so 4 queues give
    ~3x SWDGE throughput (microbenchmarked: 8.7 -> 2.9 ns/token).
  - Gathered rows are scaled by edge weights with bulk DVE multiplies
    (per-token weight broadcast along the feature axis) and scatter-added
    into per-(dst-quarter) HBM aggregation buffers, quarter q on SWDGE
    queue q, so the 4 quarters' descriptor generation also runs in
    parallel.  Duplicate dst indices inside one scatter call lose adds on
    HW, so tokens in each (batch, quarter) are grouped into
    occurrence-runs; run k goes to buffer slot (min(k,3), batch parity) -
    6 buffers per quarter - so the WAW chains between same-buffer calls
    are short and off the critical path.  Scatters of batch b are emitted
    after the gathers of batch b+1 so queue-parallel gather desc-gen is
    never head-of-line blocked by waiting scatters.
  - Class-center branch: centers = sum_cores (D_shard^T @ x) + 0.1 * cen0
    with cen0 = D^T @ x0 precomputed once during lin0, tiny [47,128]
    AllReduce (overlapped with the gather phase), then a Gram-matrix
    formulation that never materializes the [47,47,128] pair tensor.
  - Dense phases (lin0 / pass A / pass B / lin1) stream node tiles in
    13-tile chunks with one DMA per stream per chunk - per-tile DMAs made
    the Sync engine's ~0.7us/issue the phase bottleneck.

kernel(**inputs) takes the FULL unsharded inputs and returns the FULL
[100000, 47] output; sharding + all preprocessing happens on host inside.
"""

import math
import ml_dtypes
import numpy as np

from concourse import bass, bacc, tile, mybir, bass_utils
from concourse import library_config
from concourse.mybir import AxisListType
import concourse.tile_sem_assignment as _tsa
from concourse import bass_isa as _bisa

# Tile round-robins Pool-engine DMAs over all DMASW sem lanes ignoring
# queue_num; mixing SWDGE queues on one lane breaks its in-order-completion
# assumption (sim: "sem locked to SWDGE queue"). Segregate lanes by queue:
# queue q -> lanes [2q, 2q+2).
_orig_assign_tick = _tsa.TileClockTick._assign_tick

def _assign_tick_qsplit(self, inst):
    if (isinstance(inst, _tsa.DMAInst)
            and inst.engine == mybir.EngineType.Pool
            and not isinstance(inst, _bisa.UserSyncedRemoteDMADescs)
            and self.swdge_sem_count >= 8):
        qn = getattr(inst, "queue_num", 0) or 0
        if not hasattr(self, "_qrr"):
            self._qrr = {}
        r = self._qrr.get(qn, 0)
        self._qrr[qn] = r + 1
        self.next_sw_dma_idx = (qn % 4) * 2 + r % 2
    return _orig_assign_tick(self, inst)

_tsa.TileClockTick._assign_tick = _assign_tick_qsplit

F32 = mybir.dt.float32
BF16 = mybir.dt.bfloat16
I16 = mybir.dt.int16

NQ = 4          # SWDGE queues
NQUART = 4      # dst quarters (1:1 with queues for scatter)
NSLOT = 6       # agg buffer slots per quarter: (k0,p0),(k0,p1),(k1,p0),(k1,p1),k2,k3+


def _slot_of(k, b):
    ks = min(k, 3)
    if ks <= 1:
        return ks * 2 + (b % 2)
    return 2 + ks


class Cfg:
    def __init__(self, N=100000, E=800000, C=47, H=128, ncores=8, nbatch=14,
                 L=4, alpha=0.1, theta=0.5, rsl=0.5):
        self.N, self.E, self.C, self.H = N, E, C, H
        self.ncores = ncores
        self.NS = N // ncores                 # nodes per core
        self.NT = (self.NS + 127) // 128      # node tiles per core
        self.NTP = self.NT * 128              # padded rows per core
        self.Npad = self.NTP * ncores         # padded global rows
        self.nbatch = nbatch                  # src windows
        # window width over PADDED global rows (int16-safe rel idx)
        self.W = (self.Npad + nbatch - 1) // nbatch
        assert self.W + 256 < 32768
        self.NB = None                        # tokens per batch (set by prep)
        self.L, self.alpha, self.theta, self.rsl = L, alpha, theta, rsl
        # dst quarters, tile-aligned: 25/25/25/23 tiles
        qt = [25, 25, 25, self.NT - 75]
        self.q_tiles = qt
        self.q_base_tile = [0, 25, 50, 75]
        self.q_rows = [t * 128 for t in qt]          # rows per quarter
        self.q_base = [t * 128 for t in self.q_base_tile]
        # dense-phase chunks (tiles), quarter-aligned, <= CK tiles each
        self.CK = 7
        self.chunks = []
        for q in range(NQUART):
            t0 = self.q_base_tile[q]
            nt = qt[q]
            nch = (nt + self.CK - 1) // self.CK
            base, rem = divmod(nt, nch)
            off = t0
            for j in range(nch):
                sz = base + (1 if j < rem else 0)
                self.chunks.append((off, off + sz, q))
                off += sz


DEF = Cfg()


# ----------------------------------------------------------------------
# host-side edge preprocessing
# ----------------------------------------------------------------------

def _prep_edges(cfg, edge_index, edge_weight):
    """Per core: batch/window token layout, dst-quartered with occurrence runs.

    Token layout within a batch: quarters in order, within a quarter the
    occurrence-run classes in order, each (quarter, k) run padded to a
    128-token column boundary.  Shared (SPMD) static layout = max over cores.

    Returns gidx [nc, nbatch, 128, NB//16] i16 (window-relative src),
    sidx (same shape; quarter-buffer-relative dst), wgt [nc, nbatch, 128,
    NB//128] f32, runs: per batch a list of (c0, c1, quarter, slot), used:
    per batch the number of used columns, NB.
    """
    src = np.asarray(edge_index[0], np.int64)
    dst = np.asarray(edge_index[1], np.int64)
    w = np.asarray(edge_weight, np.float32)
    nc, NS, nbatch, W = cfg.ncores, cfg.NS, cfg.nbatch, cfg.W

    # src in PADDED global coordinates (x_rep rows are NTP per core)
    psrc = (src // NS) * cfg.NTP + src % NS

    per_cbq = [[[None] * NQUART for _ in range(nbatch)] for _ in range(nc)]
    kmax = 1
    for c in range(nc):
        m = (dst >= c * NS) & (dst < (c + 1) * NS)
        s_c, d_c, w_c = psrc[m], dst[m] - c * NS, w[m]
        b_c = s_c // W
        q_c = np.minimum(d_c // 3200, NQUART - 1)
        for b in range(nbatch):
            for q in range(NQUART):
                mb = (b_c == b) & (q_c == q)
                s_b, d_b, w_b = s_c[mb], d_c[mb], w_c[mb]
                order = np.argsort(d_b, kind="stable")
                s_b, d_b, w_b = s_b[order], d_b[order], w_b[order]
                occ = np.zeros(len(d_b), np.int64)
                if len(d_b):
                    is_new = np.ones(len(d_b), bool)
                    is_new[1:] = d_b[1:] != d_b[:-1]
                    grp_start = np.maximum.accumulate(
                        np.where(is_new, np.arange(len(d_b)), 0))
                    occ = np.arange(len(d_b)) - grp_start
                    kmax = max(kmax, int(occ.max()) + 1)
                per_cbq[c][b][q] = (s_b, d_b, w_b, occ)

    run_cols = np.zeros((nbatch, NQUART, kmax), np.int64)
    for b in range(nbatch):
        for q in range(NQUART):
            for k in range(kmax):
                mx = max(int((per_cbq[c][b][q][3] == k).sum())
                         for c in range(nc))
                run_cols[b, q, k] = (mx + 127) // 128

    max_cols = int(run_cols.sum(axis=(1, 2)).max())
    NB = ((max_cols + 7) // 8) * 8 * 128   # pad to 8-col gather call units
    cfg.NB = NB

    # per-batch scatter sub-calls, each <= CAP_COLS columns (SWDGE calls
    # above ~1024 tokens crash the Q7/device). Emission interleaves
    # (cap-chunk-major, then quarter) so same-buffer calls are spaced.
    CAP_COLS = 8
    runs = []
    used_cols = []
    col0 = np.zeros((nbatch, NQUART, kmax), np.int64)
    for b in range(nbatch):
        c0 = 0
        for q in range(NQUART):
            for k in range(kmax):
                n = int(run_cols[b, q, k])
                col0[b, q, k] = c0
                c0 += n
        used_cols.append(c0)
        rb = []
        for k in range(kmax):
            for s in range(0, int(run_cols[b, :, k].max()), CAP_COLS):
                for q in range(NQUART):
                    n = int(run_cols[b, q, k])
                    if s >= n:
                        continue
                    s1 = min(s + CAP_COLS, n)
                    rb.append((int(col0[b, q, k] + s),
                               int(col0[b, q, k] + s1), q, _slot_of(k, b)))
        runs.append(rb)

    gidx = np.zeros((nc, nbatch, 128, NB // 16), np.int16)
    sidx = np.zeros((nc, nbatch, 128, NB // 16), np.int16)
    wgt = np.zeros((nc, nbatch, 128, NB // 128), np.float32)
    ti = np.arange(NB)
    rows = (ti % 16)[None, :] + 16 * np.arange(8)[:, None]  # [8, NB]
    cols = ti // 16
    wrow, wcol = ti % 128, ti // 128
    for c in range(nc):
        for b in range(nbatch):
            g_lin = np.zeros(NB, np.int16)       # pads gather row 0 of window
            s_lin = np.zeros(NB, np.int16)
            w_lin = np.zeros(NB, np.float32)
            for q in range(NQUART):
                for k in range(kmax):
                    n = int(run_cols[b, q, k])
                    if n == 0:
                        continue
                    t0 = int(col0[b, q, k]) * 128
                    s_lin[t0:t0 + n * 128] = cfg.q_rows[q]   # trash default
                    s_b, d_b, w_b, occ = per_cbq[c][b][q]
                    mk = occ == k
                    cnt = int(mk.sum())
                    g_lin[t0:t0 + cnt] = (s_b[mk] - b * W).astype(np.int16)
                    s_lin[t0:t0 + cnt] = (d_b[mk] - cfg.q_base[q]).astype(
                        np.int16)
                    w_lin[t0:t0 + cnt] = w_b[mk]
            for g in range(8):
                gidx[c, b, rows[g], cols] = g_lin
                sidx[c, b, rows[g], cols] = s_lin
            wgt[c, b, wrow, wcol] = w_lin
    return gidx, sidx, wgt, runs, used_cols, NB


# ----------------------------------------------------------------------
# device program
# ----------------------------------------------------------------------

def build_nc(cfg):
    c = cfg
    nc = bacc.Bacc(None, target_bir_lowering=False, debug=False,
                   num_swdge_queues=NQ)
    NT, NB, NS, C_, H = c.NT, c.NB, c.NS, c.C, c.H
    nbw = NB // 128          # w / gtile blocks per batch
    L = c.L

    def dram_in(name, shape, dt=F32):
        return nc.declare_dram_parameter(name, shape, dt, isOutput=False)

    xin_t = dram_in("xin_t", [H, NS])
    d_t = dram_in("d_t", [NT, 128, C_], BF16)
    p_t = dram_in("p_t", [NT, C_, 128])
    gidx = dram_in("gidx", [c.nbatch, 128, NB // 16], I16)
    sidx = dram_in("sidx", [c.nbatch, 128, NB // 16], I16)
    wgt = dram_in("wgt", [c.nbatch, 128, nbw])
    lin0w = dram_in("lin0w", [H, H])
    lin0b = dram_in("lin0b", [H, 1])
    lin1w = dram_in("lin1w", [H, C_])
    lin1b = dram_in("lin1b", [C_, 1])
    convw = dram_in("convw", [L, H, H])
    cma = dram_in("cma", [C_, C_])
    cmat = dram_in("cmat", [C_, C_])
    i47 = dram_in("i47", [C_, C_])
    ident = dram_in("ident", [128, 128])
    invcnt = dram_in("invcnt", [C_, 1])
    out_t = nc.declare_dram_parameter("out_t", [C_, c.NTP], F32,
                                      isOutput=True)
    if DEBUG_DUMP:
        dbg_cen = nc.declare_dram_parameter("dbg_cen", [L, C_, H], F32,
                                            isOutput=True)
        dbg_agg = nc.declare_dram_parameter("dbg_agg", [NSLOT, 3328, H], F32,
                                            isOutput=True)
        dbg_x1 = nc.declare_dram_parameter("dbg_x1", [NT, 128, 128], F32,
                                           isOutput=True)

    # internal DRAM (node-major tensors padded to whole tiles)
    x_rep = nc.dram_tensor("x_rep", [c.Npad, H], BF16, addr_space="Shared")
    x_sh = [nc.dram_tensor(f"x_sh{i}", [c.NTP, H], BF16) for i in range(2)]
    x0_sh = nc.dram_tensor("x0_sh", [c.NTP, H], BF16)
    x_T = [nc.dram_tensor(f"x_T{i}", [NT, 128, 128], F32) for i in range(2)]
    x0_T = nc.dram_tensor("x0_T", [NT, 128, 128], F32)
    # per-quarter aggregation buffer slots, each with a trailing 128-row
    # trash pad block
    agg = [[nc.dram_tensor(f"agg{q}_{i}", [c.q_rows[q] + 128, H], BF16)
            for i in range(NSLOT)] for q in range(NQUART)]
    cen_in = nc.dram_tensor("cen_in", [C_, H], F32)
    cen_out = nc.dram_tensor("cen_out", [C_, H], F32, addr_space="Shared")

    rg = [list(range(c.ncores))]
    betas = [float(np.log(c.theta / (i + 1) + 1.0)) for i in range(L)]

    def tsize(t):
        return min(128, NS - t * 128)

    def rows_of(t0, t1):
        # node-major DRAM rows [t0*128, t1*128) viewed as [128, ct, H]
        return (t0 * 128, t1 * 128)

    with tile.TileContext(nc) as tc:
        nc.gpsimd.load_library(library_config.mlp)
        with (
            tc.tile_pool(name="const", bufs=1) as cpool,
            tc.tile_pool(name="sb", bufs=3) as pool,
            tc.tile_pool(name="ck", bufs=2) as ckpool,
            tc.tile_pool(name="gt", bufs=3) as gpool,
            tc.tile_pool(name="ps", bufs=3, space="PSUM") as psum,
            tc.tile_pool(name="psacc", bufs=1, space="PSUM") as psacc,
        ):
            # ---- resident constants ----
            lin0w_sb = cpool.tile([H, H], F32)
            nc.sync.dma_start(lin0w_sb[:], lin0w[:, :])
            lin0b_sb = cpool.tile([H, 1], F32)
            nc.sync.dma_start(lin0b_sb[:], lin0b[:, :])
            lin1w_sb = cpool.tile([H, C_], F32)
            nc.sync.dma_start(lin1w_sb[:], lin1w[:, :])
            lin1b_sb = cpool.tile([C_, 1], F32)
            nc.sync.dma_start(lin1b_sb[:], lin1b[:, :])
            convw_sb = cpool.tile([H, L * H], F32)
            for i in range(L):
                nc.sync.dma_start(convw_sb[:, i * H:(i + 1) * H], convw[i])
            cma_sb = cpool.tile([C_, C_], F32)
            nc.sync.dma_start(cma_sb[:], cma[:, :])
            cmat_sb = cpool.tile([C_, C_], F32)
            nc.sync.dma_start(cmat_sb[:], cmat[:, :])
            i47_sb = cpool.tile([C_, C_], F32)
            nc.sync.dma_start(i47_sb[:], i47[:, :])
            ident_sb = cpool.tile([128, 128], F32)
            nc.sync.dma_start(ident_sb[:], ident[:, :])
            invcnt_sb = cpool.tile([C_, 1], F32)
            nc.sync.dma_start(invcnt_sb[:], invcnt[:, :])
            zero_sb = cpool.tile([128, 1664], BF16)
            nc.vector.memset(zero_sb[:], 0.0)
            identb_sb = cpool.tile([128, 128], BF16)
            nc.vector.tensor_copy(identb_sb[:], ident_sb[:])
            cen0_sb = cpool.tile([C_, H], F32)   # D^T @ x0 (layer-invariant)

            # ---- lin0: x0 = relu(x @ W0 + b0) (chunked), write x0_T +
            #      x0_sh; also accumulate cen0 = D^T @ x0 ----
            psc0 = psacc.tile([C_, H], F32, tag="cen")
            for ci_, (t0, t1, _) in enumerate(c.chunks):
                ct = t1 - t0
                r0, r1 = rows_of(t0, t1)
                xi = ckpool.tile([H, c.CK * 128], F32, tag="xi")
                rr1 = min(r1, NS)
                nc.sync.dma_start(xi[:, :rr1 - r0], xin_t[:, r0:rr1])
                dt_ = ckpool.tile([128, c.CK, C_], BF16, tag="dt")
                nc.sync.dma_start(dt_[:, :ct, :], d_t[t0:t1].rearrange(
                    "t p c -> p t c"))
                x0ck = ckpool.tile([128, c.CK, 128], F32, tag="x0ck")
                x0nck = ckpool.tile([128, c.CK, H], BF16, tag="x0nck")
                for i in range(ct):
                    t = t0 + i
                    P = tsize(t)
                    ps0 = psum.tile([H, 128], F32, tag="b")
                    nc.tensor.matmul(ps0[:, :P], lin0w_sb[:],
                                     xi[:, i * 128:i * 128 + P],
                                     start=True, stop=True)
                    nc.scalar.activation(x0ck[:, i, :P], ps0[:, :P],
                                         mybir.ActivationFunctionType.Relu,
                                         bias=lin0b_sb[:, 0:1])
                    x0b = pool.tile([H, 128], BF16, tag="x0b")
                    nc.vector.tensor_copy(x0b[:, :P], x0ck[:, i, :P])
                    pst = psum.tile([128, 128], BF16, tag="bb")
                    nc.tensor.transpose(pst[:P, :], x0b[:, :P], identb_sb[:])
                    nc.vector.tensor_copy(x0nck[:P, i, :], pst[:P, :])
                    nc.tensor.matmul(psc0[:], dt_[:P, i, :], x0nck[:P, i, :],
                                     start=(t == 0), stop=(t == NT - 1))
                nc.sync.dma_start(x0_T[t0:t1].rearrange("t p n -> p t n"),
                                  x0ck[:, :ct, :])
                nc.sync.dma_start(
                    x0_sh[r0:r1, :].rearrange("(t p) h -> p t h", p=128),
                    x0nck[:, :ct, :])
            nc.vector.tensor_copy(cen0_sb[:], psc0[:])

            # ---- layers ----
            for li in range(L):
                beta = betas[li]
                cur_sh = x0_sh if li == 0 else x_sh[li % 2]
                cur_T = x0_T if li == 0 else x_T[li % 2]
                nxt_sh = x_sh[(li + 1) % 2]
                nxt_T = x_T[(li + 1) % 2]

                # - pass A: centers partial (chunked; emitted before the
                #   AllGather so its work overlaps the collective) -
                psA = psacc.tile([C_, H], F32, tag="cen")
                for (t0, t1, _) in c.chunks:
                    ct = t1 - t0
                    r0, r1 = rows_of(t0, t1)
                    xa = ckpool.tile([128, c.CK, H], BF16, tag="xa")
                    nc.sync.dma_start(
                        xa[:, :ct, :],
                        cur_sh[r0:r1, :].rearrange("(t p) h -> p t h", p=128))
                    dt_ = ckpool.tile([128, c.CK, C_], BF16, tag="dt")
                    nc.sync.dma_start(dt_[:, :ct, :], d_t[t0:t1].rearrange(
                        "t p c -> p t c"))
                    for i in range(ct):
                        t = t0 + i
                        P = tsize(t)
                        nc.tensor.matmul(psA[:], dt_[:P, i, :], xa[:P, i, :],
                                         start=(t == 0), stop=(t == NT - 1))
                cenp = pool.tile([C_, H], F32, tag="cenp")
                nc.vector.scalar_tensor_tensor(
                    cenp[:], cen0_sb[:], 0.1, psA[:],
                    op0=mybir.AluOpType.mult, op1=mybir.AluOpType.add)
                nc.sync.dma_start(cen_in[:, :], cenp[:])

                # - allgather x -
                nc.gpsimd.collective_compute(
                    "AllGather", mybir.AluOpType.bypass, replica_groups=rg,
                    ins=[cur_sh.ap().opt()], outs=[x_rep.ap().opt()],
                )
                # - centers allreduce (overlaps the gather phase) -
                nc.gpsimd.collective_compute(
                    "AllReduce", mybir.AluOpType.add, replica_groups=rg,
                    ins=[cen_in.ap().opt()], outs=[cen_out.ap().opt()],
                )

                # - zero agg buffers (overlaps the collectives) -
                for q in range(NQUART):
                    for a in range(NSLOT):
                        nrow = c.q_rows[q] + 128
                        step = 1664
                        for r0 in range(0, nrow, step):
                            r1 = min(r0 + step, nrow)
                            nc.sync.dma_start(agg[q][a][r0:r1, :],
                                              zero_sb[:, :r1 - r0])

                # - sparse propagate; scatters of batch b are interleaved
                #   between the gathers of batch b+2 (2-batch software
                #   pipeline) so waiting scatters never head-of-line block
                #   queue-parallel gather desc-gen in the Pool engine's
                #   4-deep wait queue -
                LAG = 2
                gts = {}
                sis = {}
                for b in range(c.nbatch + LAG):
                    gcalls = []
                    if b < c.nbatch:
                        base = b * c.W
                        wrows = min(c.W + 256, c.Npad - base)
                        used = USED[b]
                        gi = pool.tile([128, NB // 16], I16, tag="gi")
                        nc.sync.dma_start(gi[:], gidx[b])
                        si = pool.tile([128, NB // 16], I16, tag="si")
                        nc.sync.dma_start(si[:], sidx[b])
                        wt = pool.tile([128, nbw], F32, tag="wt")
                        nc.sync.dma_start(wt[:], wgt[b])
                        gtile = gpool.tile([128, nbw, H], BF16, tag="g")
                        gts[b] = gtile
                        sis[b] = si
                        for gci, g0 in enumerate(range(0, used, 8)):
                            g1 = min(g0 + 8, nbw)
                            gcalls.append((gtile, base, wrows, gi, wt,
                                           gci, g0, g1, b))
                    scalls = RUNS[b - LAG] if b >= LAG else []
                    gtile_p = gts.pop(b - LAG, None)
                    si_p = sis.pop(b - LAG, None)
                    # interleave: 1 gather then ~3 scatters, repeating
                    gi_i, si_i = 0, 0
                    while gi_i < len(gcalls) or si_i < len(scalls):
                        if gi_i < len(gcalls):
                            (gtile_, base_, wrows_, gitile, wtile,
                             gci, g0, g1, bb) = gcalls[gi_i]
                            gi_i += 1
                            ntok = (g1 - g0) * 128
                            nc.gpsimd.dma_gather(
                                gtile_[:, g0:g1, :],
                                x_rep[base_:base_ + wrows_, :],
                                gitile[:, g0 * 8:g1 * 8],
                                num_idxs=ntok, num_idxs_reg=ntok,
                                elem_size=H, queue_num=(bb + gci) % NQ,
                            )
                            wt3 = wtile[:, g0:g1].unsqueeze(2).broadcast_to(
                                (128, g1 - g0, H))
                            nc.vector.tensor_mul(gtile_[:, g0:g1, :],
                                                 gtile_[:, g0:g1, :], wt3)
                        for _ in range(3):
                            if si_i >= len(scalls):
                                break
                            (c0, c1, q, slot) = scalls[si_i]
                            si_i += 1
                            ntok = (c1 - c0) * 128
                            nc.gpsimd.dma_scatter_add(
                                agg[q][slot][:, :], gtile_p[:, c0:c1, :],
                                si_p[:, c0 * 8:c1 * 8],
                                num_idxs=ntok, num_idxs_reg=ntok,
                                elem_size=H, queue_num=q,
                            )

                # - r_cls from centers (Gram trick); d_t is an exact one-hot
                #   so the f32 1/cnt scale happens here, AFTER the reduce -
                cenr = pool.tile([C_, H], F32, tag="cenr")
                nc.sync.dma_start(cenr[:], cen_out[:, :])
                cen = pool.tile([C_, H], F32, tag="cen_sb")
                nc.vector.tensor_scalar(cen[:], cenr[:], invcnt_sb[:, 0:1],
                                        None, mybir.AluOpType.mult)
                if DEBUG_DUMP:
                    nc.sync.dma_start(dbg_cen[li], cen[:])
                    if li == 0:
                        for slot in range(NSLOT):
                            for r0 in range(0, 3328, c.CK * 128):
                                da = pool.tile([128, c.CK, H], BF16, tag="dbgda")
                                nc.sync.dma_start(
                                    da[:, :min(c.CK*128, 3328-r0)//128, :],
                                    agg[0][slot][r0:r0 + min(c.CK*128, 3328-r0), :].rearrange(
                                        "(t p) h -> p t h", p=128))
                                daf = pool.tile([128, c.CK, H], F32, tag="dbgdf")
                                nc.vector.tensor_copy(daf[:], da[:])
                                nc.sync.dma_start(
                                    dbg_agg[slot][r0:r0 + 1664, :].rearrange(
                                        "(t p) h -> p t h", p=128), daf[:])
                pst = psum.tile([128, C_], F32, tag="b")
                nc.tensor.transpose(pst[:, :], cen[:], ident_sb[:C_, :C_])
                cT = pool.tile([128, C_], F32, tag="cT")
                nc.vector.tensor_copy(cT[:], pst[:, :])
                psg = psum.tile([C_, C_], F32, tag="b")
                nc.tensor.matmul(psg[:], cT[:], cT[:], start=True, stop=True)
                g = pool.tile([C_, C_], F32, tag="gg")
                nc.vector.tensor_copy(g[:], psg[:])
                gd = pool.tile([C_, C_], F32, tag="gd")
                nc.vector.tensor_mul(gd[:], g[:], i47_sb[:])
                n2 = pool.tile([C_, 1], F32, tag="n2")
                nc.vector.reduce_sum(n2[:], gd[:], AxisListType.X)
                t1_ = pool.tile([C_, C_], F32, tag="t1")
                nc.vector.tensor_scalar(t1_[:], g[:], -1.0, n2[:, 0:1],
                                        mybir.AluOpType.mult,
                                        mybir.AluOpType.add)
                ps1 = psum.tile([C_, C_], F32, tag="b")
                nc.tensor.transpose(ps1[:], t1_[:], ident_sb[:C_, :C_])
                nrm = pool.tile([C_, C_], F32, tag="nrm")
                nc.vector.tensor_add(nrm[:], t1_[:], ps1[:])
                nc.vector.tensor_relu(nrm[:], nrm[:])
                nc.vector.tensor_add(nrm[:], nrm[:], i47_sb[:])
                rn = pool.tile([C_, C_], F32, tag="rn")
                nc.scalar.sqrt(rn[:], nrm[:])
                inv = pool.tile([C_, C_], F32, tag="inv")
                nc.vector.reciprocal(inv[:], rn[:])
                amat = pool.tile([C_, C_], F32, tag="amat")
                nc.vector.tensor_mul(amat[:], cma_sb[:], inv[:])
                atm = pool.tile([C_, C_], F32, tag="atm")
                nc.vector.tensor_mul(atm[:], cmat_sb[:], inv[:])
                rs = pool.tile([C_, 1], F32, tag="rs")
                nc.vector.reduce_sum(rs[:], amat[:], AxisListType.X)
                psm = psum.tile([C_, H], F32, tag="b")
                nc.tensor.matmul(psm[:], atm[:], cen[:], start=True, stop=True)
                rcls = pool.tile([C_, H], F32, tag="rcls")
                nc.vector.tensor_scalar(rcls[:], cen[:], rs[:, 0:1], None,
                                        mybir.AluOpType.mult)
                nc.vector.tensor_sub(rcls[:], rcls[:], psm[:])

                # - pass B (chunked) -
                for (t0, t1, q) in c.chunks:
                    ct = t1 - t0
                    r0, r1 = rows_of(t0, t1)
                    lr0 = t0 * 128 - c.q_base[q]
                    # sum the quarter's agg slots for this chunk
                    asum = ckpool.tile([128, c.CK, H], F32, tag="asum")
                    for slot in range(NSLOT):
                        ast = pool.tile([128, c.CK, H], BF16, tag="ast")
                        nc.sync.dma_start(
                            ast[:, :ct, :],
                            agg[q][slot][lr0:lr0 + ct * 128, :].rearrange(
                                "(t p) h -> p t h", p=128))
                        if slot == 0:
                            nc.vector.tensor_copy(asum[:, :ct, :],
                                                  ast[:, :ct, :])
                        else:
                            nc.vector.tensor_add(asum[:, :ct, :],
                                                 asum[:, :ct, :],
                                                 ast[:, :ct, :])
                    pt = ckpool.tile([C_, c.CK, 128], F32, tag="pt")
                    nc.sync.dma_start(pt[:, :ct, :], p_t[t0:t1].rearrange(
                        "t c n -> c t n"))
                    xt = ckpool.tile([128, c.CK, 128], F32, tag="xt")
                    nc.sync.dma_start(xt[:, :ct, :], cur_T[t0:t1].rearrange(
                        "t h n -> h t n"))
                    x0t2 = ckpool.tile([128, c.CK, 128], F32, tag="x0t2")
                    nc.sync.dma_start(x0t2[:, :ct, :], x0_T[t0:t1].rearrange(
                        "t h n -> h t n"))
                    xnck = ckpool.tile([128, c.CK, 128], F32, tag="xnck")
                    xnsh = ckpool.tile([128, c.CK, H], BF16, tag="xnsh")
                    for i in range(ct):
                        t = t0 + i
                        P = tsize(t)
                        ps1b = psum.tile([H, 128], F32, tag="b")
                        nc.tensor.matmul(ps1b[:, :P], rcls[:], pt[:, i, :P],
                                         start=True, stop=True)
                        psT = psum.tile([H, 128], F32, tag="b")
                        nc.tensor.transpose(psT[:, :P], asum[:P, i, :],
                                            ident_sb[:P, :P])
                        # u = 0.45*(x + aggT + r) + 0.1*x0
                        s1 = pool.tile([H, 128], F32, tag="s1")
                        nc.vector.tensor_add(s1[:, :P], xt[:, i, :P],
                                             psT[:, :P])
                        nc.vector.tensor_add(s1[:, :P], s1[:, :P],
                                             ps1b[:, :P])
                        ux = pool.tile([H, 128], F32, tag="ux")
                        nc.vector.tensor_scalar(ux[:, :P], x0t2[:, i, :P],
                                                0.1, None,
                                                mybir.AluOpType.mult)
                        u = pool.tile([H, 128], F32, tag="u")
                        nc.vector.scalar_tensor_tensor(
                            u[:, :P], s1[:, :P], 0.45, ux[:, :P],
                            op0=mybir.AluOpType.mult, op1=mybir.AluOpType.add)
                        ps2 = psum.tile([H, 128], F32, tag="b")
                        nc.tensor.matmul(ps2[:, :P],
                                         convw_sb[:, li * H:(li + 1) * H],
                                         u[:, :P], start=True, stop=True)
                        # x_next = relu((1-b) * (u + (b/(1-b))*(u@W)))
                        t3 = pool.tile([H, 128], F32, tag="t3")
                        nc.vector.scalar_tensor_tensor(
                            t3[:, :P], ps2[:, :P], beta / (1.0 - beta),
                            u[:, :P], op0=mybir.AluOpType.mult,
                            op1=mybir.AluOpType.add)
                        nc.scalar.activation(xnck[:, i, :P], t3[:, :P],
                                             mybir.ActivationFunctionType.Relu,
                                             scale=1.0 - beta)
                        xnb = pool.tile([H, 128], BF16, tag="xnb")
                        nc.vector.tensor_copy(xnb[:, :P], xnck[:, i, :P])
                        psn = psum.tile([128, 128], BF16, tag="bb")
                        nc.tensor.transpose(psn[:P, :], xnb[:, :P],
                                            identb_sb[:])
                        nc.vector.tensor_copy(xnsh[:P, i, :], psn[:P, :])
                    nc.sync.dma_start(nxt_T[t0:t1].rearrange(
                        "t h n -> h t n"), xnck[:, :ct, :])
                    nc.sync.dma_start(
                        nxt_sh[r0:r1, :].rearrange("(t p) h -> p t h", p=128),
                        xnsh[:, :ct, :])
                    if DEBUG_DUMP and li == 0:
                        nc.sync.dma_start(dbg_x1[t0:t1].rearrange(
                            "t h n -> h t n"), xnck[:, :ct, :])

            # ---- lin1 (chunked) ----
            fin_T = x_T[L % 2]
            for (t0, t1, _) in c.chunks:
                ct = t1 - t0
                r0, r1 = rows_of(t0, t1)
                xt = ckpool.tile([128, c.CK, 128], F32, tag="fxt")
                nc.sync.dma_start(xt[:, :ct, :], fin_T[t0:t1].rearrange(
                    "t h n -> h t n"))
                otck = ckpool.tile([C_, c.CK, 128], F32, tag="ot")
                for i in range(ct):
                    t = t0 + i
                    P = tsize(t)
                    psf = psum.tile([C_, 128], F32, tag="b")
                    nc.tensor.matmul(psf[:, :P], lin1w_sb[:], xt[:, i, :P],
                                     start=True, stop=True)
                    nc.vector.tensor_scalar(otck[:, i, :P], psf[:, :P],
                                            lin1b_sb[:, 0:1], None,
                                            mybir.AluOpType.add)
                nc.sync.dma_start(out_t[:, r0:r1].rearrange(
                    "c (t n) -> c t n", n=128), otck[:, :ct, :])

    nc.compile()
    return nc


# RUNS / USED are read by build_nc (static layout shared across cores)
RUNS = None
USED = None
DEBUG_DUMP = False


# ----------------------------------------------------------------------
# host wrapper
# ----------------------------------------------------------------------

def _prep_inputs(cfg, inputs):
    c = cfg
    x = np.asarray(inputs["x"], np.float32)
    label = np.asarray(inputs["label"], np.int64)
    p = np.asarray(inputs["p"], np.float32)
    cm = np.asarray(inputs["cm"], np.float32)
    lin0_w = np.asarray(inputs["lin0_w"], np.float32)
    lin0_b = np.asarray(inputs["lin0_b"], np.float32)
    lin1_w = np.asarray(inputs["lin1_w"], np.float32)
    lin1_b = np.asarray(inputs["lin1_b"], np.float32)
    conv_w = np.asarray(inputs["conv_w"], np.float32)

    gidx, sidx, wgt, runs, used, NB = _prep_edges(
        cfg, inputs["edge_index"], inputs["edge_weight"])

    cnt = np.bincount(label, minlength=c.C).astype(np.float32)
    cnt = np.maximum(cnt, 1.0)
    NTP = c.NT * 128
    cma = cm[:, 0, :]
    i47 = np.eye(c.C, dtype=np.float32)
    ident = np.eye(128, dtype=np.float32)

    in_maps = []
    for ci in range(c.ncores):
        r0 = ci * c.NS
        xs = x[r0:r0 + c.NS]                      # [NS, H]
        lab = label[r0:r0 + c.NS]
        ps = p[r0:r0 + c.NS]                      # [NS, C]
        d_t = np.zeros((NTP, c.C), np.float32)
        d_t[np.arange(c.NS), lab] = 1.0
        p_pad = np.zeros((NTP, c.C), np.float32)
        p_pad[:c.NS] = ps
        in_maps.append({
            "xin_t": np.ascontiguousarray(xs.T),
            "d_t": np.ascontiguousarray(
                d_t.reshape(c.NT, 128, c.C)).astype(ml_dtypes.bfloat16),
            "p_t": np.ascontiguousarray(
                p_pad.reshape(c.NT, 128, c.C).transpose(0, 2, 1)),
            "gidx": gidx[ci], "sidx": sidx[ci], "wgt": wgt[ci],
            "lin0w": lin0_w, "lin0b": lin0_b.reshape(-1, 1),
            "lin1w": lin1_w, "lin1b": lin1_b.reshape(-1, 1),
            "convw": conv_w, "cma": cma,
            "cmat": np.ascontiguousarray(cma.T),
            "i47": i47, "ident": ident,
            "invcnt": (1.0 / cnt).reshape(-1, 1).astype(np.float32),
        })
    return in_maps, runs, used


_BUILT = {}


def kernel(**inputs):
    cfg = DEF
    global RUNS, USED
    in_maps, runs, used = _prep_inputs(cfg, inputs)
    key = "default"
    if key not in _BUILT:
        RUNS = runs
        USED = used
        _BUILT[key] = build_nc(cfg)
    nc = _BUILT[key]
    res = bass_utils.run_bass_kernel_spmd(nc, in_maps,
                                          core_ids=list(range(cfg.ncores)))
    outs = [res.results[ci]["out_t"].T[:cfg.NS] for ci in range(cfg.ncores)]
    return np.ascontiguousarray(np.concatenate(outs, 0))
